# revision 29
# baseline (speedup 1.0000x reference)
"""LSTM greedy decoder on 8 trn2 NeuronCores.

Vocab-parallel: each core keeps a resident fp32r SBUF copy of its
4000-row W_out shard, replicates the LSTM cell (exact fp32), and agrees
on the greedy token by exchanging per-core top-2 exact candidates via a
tiny AllGather.

Scheduling notes (r1):
- per-tile MAX8/MAX_INDEX8 run pipelined under the logits matmuls and
  write straight into persistent [B, NT, 8] slots (no per-tile copies).
- the b_out bias-add runs on GpSimd reading the PSUM tile directly and
  writing the SBUF logits tile (the scalar PSUM->SBUF copy is gone).
- the top-2 candidate logits are re-evaluated exactly in fp32 BEFORE
  the AllGather; each candidate's W_out row gather is issued as soon as
  its id is known (overlapping the next selection round), and each dot
  is one fused tensor_tensor_reduce.
- AllGather staging uses DVE 32x32 stream-transposes + contiguous DMAs.
- the whe token-row gather is split in two column halves ([f,g] then
  [i,o]) so the LSTM pointwise can start on the first half early.
- LSTM pointwise is choreographed across engines: per-gate adds on
  Vector (+o on GpSimd), activations on Scalar in dependency order
  (sigmoid f, tanh g, sigmoid i, sigmoid o), fc on GpSimd.
- gate layout is host-permuted to [f, g, i, o].
- gates matmuls for step t+1 are emitted after the logits so the PE
  works under the collective tail.
- dense PE warm bursts (dummy transposes reading constants, so no data
  deps) keep the PE p-state high across the pointwise window.
- all single-buffer tail scratch tiles are allocated once, outside the
  step loop.
"""

import numpy as np

B, H, D, V, S = 64, 512, 256, 32000, 64
NCORES = 8
VS = V // NCORES            # 4000 vocab rows per core
G4 = 4 * H                  # 2048 gate units
NT = 8                      # logits N-tiles per step
TN = VS // NT               # 500 columns per logits tile
KH = H // 128               # 4 contraction tiles over H
BIG = 1.0e9
NCAND = 2                   # exact-rechecked candidates per core
TPT = 3                     # per-tile candidates pooled


def build_program(steps=S):
    import concourse.bass as bass
    import concourse.bacc as bacc
    import concourse.mybir as mybir
    import concourse.tile as tile
    from concourse.masks import make_identity

    f32 = mybir.dt.float32
    bf16 = mybir.dt.bfloat16
    u32 = mybir.dt.uint32
    AF = mybir.ActivationFunctionType
    OP = mybir.AluOpType
    AX = mybir.AxisListType

    nc = bacc.Bacc(num_devices=NCORES)
    h0T_p = nc.declare_dram_parameter("h0T", [H, B], f32, isOutput=False)
    c0_p = nc.declare_dram_parameter("c0", [B, H], f32, isOutput=False)
    whhT_p = nc.declare_dram_parameter("whhT", [H, G4], f32, isOutput=False)
    bias_p = nc.declare_dram_parameter("bias", [1, G4], f32, isOutput=False)
    woutT_p = nc.declare_dram_parameter("woutT", [H, VS], f32, isOutput=False)
    bout_p = nc.declare_dram_parameter("bout", [1, VS], f32, isOutput=False)
    whe0_p = nc.declare_dram_parameter("whe0", [V, G4 // 2], f32, isOutput=False)
    whe1_p = nc.declare_dram_parameter("whe1", [V, G4 // 2], f32, isOutput=False)
    wfullb_p = nc.declare_dram_parameter("wfullb", [V, H + 1], f32, isOutput=False)
    tbase_p = nc.declare_dram_parameter("tbase", [B, NT * TPT], f32, isOutput=False)
    out_p = nc.declare_dram_parameter("out", [steps, B, VS], bf16, isOutput=True)

    rg = [list(range(NCORES))]
    HG = G4 // 2  # 1024: [f,g] | [i,o] halves
    sF = slice(0, 512)
    sG = slice(512, 1024)
    sI = slice(1024, 1536)
    sO = slice(1536, 2048)

    with tile.TileContext(nc) as tc:
        with (
            tc.tile_pool(name="wpool", bufs=1) as wp,
            tc.tile_pool(name="state", bufs=2) as sp,
            tc.tile_pool(name="work", bufs=2) as kp,
            tc.tile_pool(name="ps_g", bufs=1, space="PSUM") as pg,
            tc.tile_pool(name="ps_l", bufs=2, space="PSUM") as pl,
            tc.tile_pool(name="ps_t", bufs=2, space="PSUM") as pt,
            tc.tile_pool(name="dram", bufs=2, space="DRAM") as dp,
        ):
            # ---- constants (engine-local, no DMA) ----
            ident = wp.tile([128, 128], f32)
            make_identity(nc, ident[:])
            ones1 = wp.tile([1, B], f32)
            nc.vector.memset(ones1[:], 1.0)

            # ---- resident weights (barriers cap per-inst sync-wait fan-in) ----
            tc.strict_bb_all_engine_barrier()
            whh = wp.tile([128, KH, G4], f32)
            nc.sync.dma_start(out=whh[:], in_=whhT_p[:].rearrange("(a p) n -> p a n", p=128))
            bias = wp.tile([1, G4], f32)
            nc.sync.dma_start(out=bias[:], in_=bias_p[:])
            tbase = wp.tile([B, NT * TPT], f32)
            nc.sync.dma_start(out=tbase[:], in_=tbase_p[:])
            tc.strict_bb_all_engine_barrier()

            # bf16 logits weights: stage fp32 chunks through the logits-tagged
            # slot, then round-copy. bf16 matmul runs 1 cyc/row (vs fp32r's
            # effective 2) and at ~half the PE power, which also relieves the
            # HAM util-limit throttle windows. The exact fp32 recheck of the
            # top-2 candidates keeps the token feedback path exact.
            wout = wp.tile([128, KH, VS], bf16)
            wq = woutT_p[:].rearrange("(a p) n -> p a n", p=128)
            for k in range(KH):
                stage = kp.tile([128, VS], f32, tag="logits", name=f"wstage{k}")
                nc.sync.dma_start(out=stage[:], in_=wq[:, k, :])
                nc.vector.tensor_copy(wout[:, k, :], stage[:])
                tc.strict_bb_all_engine_barrier()
            # broadcast b_out across the 64 batch partitions once
            biasb = wp.tile([B, VS], f32)
            bstage = kp.tile([1, VS], f32, tag="logits", name="bstage")
            nc.sync.dma_start(out=bstage[:], in_=bout_p[:])
            for n in range(NT):
                ns = slice(n * TN, (n + 1) * TN)
                lg = pl.tile([B, TN], f32, tag="lg")
                nc.tensor.matmul(out=lg[:], lhsT=ones1[:], rhs=bstage[:, ns],
                                 start=True, stop=True)
                nc.scalar.activation(biasb[:, ns], lg[:], AF.Copy)
            tc.strict_bb_all_engine_barrier()

            # ---- loop-invariant scratch (alloc once: per-iter bufs=1
            # re-allocs hit the min-join fallback and stall on Tensor) ----
            pk = wp.tile([B, 32], f32)          # [ev0 ev1 | id0 id1 | pad]
            nc.vector.memset(pk[:], 0.0)
            pkT = wp.tile([B, 32], f32)
            gpre = wp.tile([B, 32], f32)
            nc.vector.memset(gpre[:], 0.0)
            gpT = wp.tile([B, 32], f32)
            m8all = wp.tile([B, NT, 8], f32)    # per-tile sorted top-8 values
            i8all = wp.tile([B, NT, 8], u32)    # per-tile top-8 indices
            tif = wp.tile([B, NT, TPT], f32)    # pooled candidate global ids
            m8l = wp.tile([B, 8], f32)
            si = wp.tile([B, NCAND], f32)
            dcand = wp.tile([B, NT, TPT], f32)
            dhi = wp.tile([B, NT, TPT], f32)
            cu = wp.tile([B, NCAND], u32)
            wrow3 = wp.tile([B, NCAND, H + 1], f32)
            prod3 = wp.tile([B, NCAND, H + 1], f32)
            gm8 = wp.tile([B, 8], f32)
            msk = wp.tile([B, NCORES, NCAND], f32)
            gidxf = wp.tile([B, 1], f32)
            gidx = wp.tile([B, 1], u32)
            gx = wp.tile([B, G4], f32)
            nc.vector.memset(gx[:], 0.0)
            gsb = wp.tile([B, G4], f32)
            acts = wp.tile([B, G4], f32)
            fc = wp.tile([B, H], f32)
            ig = wp.tile([B, H], f32)
            tct = wp.tile([B, H], f32)
            h1a = wp.tile([B, H + 1], f32)
            nc.vector.memset(h1a[:, H : H + 1], 1.0)
            h1 = h1a[:, 0:H]

            hT = sp.tile([128, KH, B], f32, tag="hT")
            nc.sync.dma_start(out=hT[:], in_=h0T_p[:].rearrange("(a p) b -> p a b", p=128))
            tc.strict_bb_all_engine_barrier()
            hTr = sp.tile([128, KH, B], bf16, tag="hTr")
            nc.vector.tensor_copy(hTr[:], hT[:])
            c_prev = sp.tile([B, H], f32, tag="c")
            nc.sync.dma_start(out=c_prev[:], in_=c0_p[:])
            tc.strict_bb_all_engine_barrier()

            # ---- gates for t=0: bias + W_hh @ h0 (exact fp32) ----
            gates = pg.tile([B, G4], f32, tag="gates")
            for n in range(4):
                ns = slice(n * 512, (n + 1) * 512)
                nc.tensor.matmul(out=gates[:, ns], lhsT=ones1[:], rhs=bias[:, ns],
                                 start=True, stop=False)
                for k in range(KH):
                    nc.tensor.matmul(out=gates[:, ns], lhsT=hT[:, k, :], rhs=whh[:, k, ns],
                                     start=False, stop=(k == KH - 1))

            first = True
            for t in range(steps):
                # ---- LSTM pointwise; gate layout is [f, g, i, o] ----
                # per-gate source: t=0 reads gates PSUM directly (no x-part);
                # later steps add the gathered x-contribution first.
                if first:
                    src = gates
                else:
                    nc.vector.tensor_tensor(gsb[:, sF], gates[:, sF], gx[:, sF], op=OP.add)
                    nc.vector.tensor_tensor(gsb[:, sG], gates[:, sG], gx[:, sG], op=OP.add)
                    nc.vector.tensor_tensor(gsb[:, sI], gates[:, sI], gx[:, sI], op=OP.add)
                    nc.vector.tensor_tensor(gsb[:, sO], gates[:, sO], gx[:, sO], op=OP.add)
                    src = gsb
                nc.scalar.activation(acts[:, sF], src[:, sF], AF.Sigmoid)
                nc.scalar.activation(acts[:, sG], src[:, sG], AF.Tanh)
                nc.scalar.activation(acts[:, sI], src[:, sI], AF.Sigmoid)
                nc.scalar.activation(acts[:, sO], src[:, sO], AF.Sigmoid)
                first = False
                # dense PE warm burst with deps staggered through the
                # pointwise window (gx -> acts -> tct) so the PE stays near
                # its high p-state right up to the hT transposes
                for w in range(4):
                    wmt = pt.tile([128, B], f32, tag="tp")
                    nc.tensor.transpose(out=wmt[:], in_=gx[:, 128 * w : 128 * (w + 1)],
                                        identity=ident[0:B, 0:B])
                for src_ap in (acts[:, 0:128], acts[:, 512:640], acts[:, 1024:1152],
                               acts[:, 1536:1664]):
                    wmt = pt.tile([128, B], f32, tag="tp")
                    nc.tensor.transpose(out=wmt[:], in_=src_ap, identity=ident[0:B, 0:B])
                nc.gpsimd.tensor_tensor(fc[:], acts[:, sF], c_prev[:], op=OP.mult)
                nc.vector.tensor_tensor(ig[:], acts[:, sI], acts[:, sG], op=OP.mult)
                c_new = sp.tile([B, H], f32, tag="c")
                nc.vector.tensor_tensor(c_new[:], fc[:], ig[:], op=OP.add)
                nc.scalar.activation(tct[:], c_new[:], AF.Tanh)
                for w in range(2):
                    wmt = pt.tile([128, B], f32, tag="tp")
                    nc.tensor.transpose(out=wmt[:], in_=tct[:, 128 * w : 128 * (w + 1)],
                                        identity=ident[0:B, 0:B])
                nc.vector.tensor_tensor(h1[:], acts[:, sO], tct[:], op=OP.mult)
                c_prev = c_new

                # ---- h1 -> h1.T tiles (PE transpose); both copies on DVE so
                # the logits (hTr) unblock no later than the gates (hT) ----
                hT = sp.tile([128, KH, B], f32, tag="hT")
                hTr = sp.tile([128, KH, B], bf16, tag="hTr")
                for k in range(KH):
                    tp = pt.tile([128, B], f32, tag="tp")
                    nc.tensor.transpose(out=tp[:], in_=h1[:, k * 128 : (k + 1) * 128],
                                        identity=ident[0:B, 0:B])
                    nc.vector.tensor_copy(hTr[:, k, :], tp[:])
                    nc.vector.tensor_copy(hT[:, k, :], tp[:])

                # ---- logits shard in bf16 + per-tile top-8 scan ----
                # scalar drains PSUM -> SBUF, GpSimd adds b_out in place,
                # MAX8/MAX_INDEX8 on the biased f32 SBUF tile (exact f32
                # values, so no duplicate-tie hazard); a second scalar copy
                # mirrors each tile to bf16 so the output write is half-size
                logits = kp.tile([B, VS], f32, tag="logits")
                outb = kp.tile([B, VS], bf16, tag="outb")
                last = t == steps - 1
                for n in range(NT):
                    ns = slice(n * TN, (n + 1) * TN)
                    lg = pl.tile([B, TN], f32, tag="lg")
                    for k in range(KH):
                        nc.tensor.matmul(out=lg[:], lhsT=hTr[:, k, :], rhs=wout[:, k, ns],
                                         start=(k == 0), stop=(k == KH - 1))
                    nc.scalar.activation(logits[:, ns], lg[:], AF.Copy)
                    nc.gpsimd.tensor_tensor(logits[:, ns], logits[:, ns], biasb[:, ns], op=OP.add)
                    nc.scalar.activation(outb[:, ns], logits[:, ns], AF.Copy)
                    if not last:
                        nc.vector.max(out=m8all[:, n, :], in_=logits[:, ns])
                        nc.vector.max_index(out=i8all[:, n, :], in_max=m8all[:, n, :],
                                            in_values=logits[:, ns])
                if last:
                    nc.scalar.dma_start(out=out_p[t], in_=outb[:])
                    break

                # ---- gates h-part for t+1 (exact fp32) — PE works under the tail ----
                gates = pg.tile([B, G4], f32, tag="gates")
                for n in range(4):
                    ns = slice(n * 512, (n + 1) * 512)
                    for k in range(KH):
                        nc.tensor.matmul(out=gates[:, ns], lhsT=hT[:, k, :], rhs=whh[:, k, ns],
                                         start=(k == 0), stop=(k == KH - 1))

                # ---- local top-2 (distinct ids) from the 24 pooled candidates;
                # each candidate's W_out row gather is issued as soon as its id
                # is known so the transfer overlaps the next selection round ----
                # top-2 by the sorted pooled values: slot k keeps entries in
                # the value band [m8l[k], m8l[k-1]) via (v is_lt m8l[k])*BIG +
                # (v is_ge m8l[k-1])*BIG + id, then min-reduce. Both chains
                # depend only on m8l, so the candidate-0 row gather issues
                # while the candidate-1 selection still runs. (Exact f32 value
                # ties across distinct ids are ~impossible.)
                tv = m8all[:, :, 0:TPT]
                nc.vector.tensor_copy(tif[:], i8all[:, :, 0:TPT])
                nc.vector.tensor_tensor(tif[:], tif[:], tbase[:].rearrange("b (n c) -> b n c", c=TPT), op=OP.add)
                nc.vector.max(out=m8l[:], in_=tv)
                for k in range(NCAND):
                    nc.vector.tensor_scalar(dcand[:], tv, m8l[:, k : k + 1], BIG, op0=OP.is_lt, op1=OP.mult)
                    if k > 0:
                        nc.vector.tensor_scalar(dhi[:], tv, m8l[:, k - 1 : k], BIG, op0=OP.is_ge, op1=OP.mult)
                        nc.vector.tensor_tensor(dcand[:], dcand[:], dhi[:], op=OP.add)
                    nc.vector.tensor_tensor(dcand[:], tif[:], dcand[:], op=OP.add)
                    nc.vector.tensor_reduce(si[:, k : k + 1], dcand[:], axis=AX.XY, op=OP.min)
                    nc.vector.tensor_copy(cu[:, k : k + 1], si[:, k : k + 1])
                    nc.gpsimd.indirect_dma_start(
                        out=wrow3[:, k], out_offset=None, in_=wfullb_p[:],
                        in_offset=bass.IndirectOffsetOnAxis(ap=cu[:, k : k + 1], axis=0),
                    )
                nc.vector.tensor_copy(pk[:, NCAND : 2 * NCAND], si[:])

                # ---- exact fp32 recheck of the candidates ----
                for j in range(NCAND):
                    nc.vector.tensor_tensor(prod3[:, j], wrow3[:, j], h1a[:], op=OP.mult)
                nc.vector.tensor_reduce(pk[:, 0:NCAND], prod3[:], axis=AX.X, op=OP.add)

                # ---- AllGather of (2 exact vals | 2 ids) per core ----
                nc.vector.transpose(pkT[:], pk[:])
                agin = dp.tile([2 * NCAND, B], f32, tag="agin")
                nc.sync.dma_start(out=agin[:, 0:32], in_=pkT[0 : 2 * NCAND, 0:32])
                nc.scalar.dma_start(out=agin[:, 32:64], in_=pkT[32 : 32 + 2 * NCAND, 0:32])
                # the 0.5MB bf16 logits write is issued here so most of its
                # transfers ride inside the collective window instead of
                # contending with the recheck row gathers
                nc.sync.dma_start(out=out_p[t], in_=outb[:])
                agout = dp.tile([NCORES * 2 * NCAND, B], f32, tag="agout", addr_space="Shared")
                nc.gpsimd.collective_compute(
                    "AllGather", OP.bypass, replica_groups=rg,
                    ins=[agin[:].opt()], outs=[agout[:].opt()],
                )
                # block-swapped readback so one DVE stream-transpose finishes it
                nc.sync.dma_start(out=gpre[0:32, 0:32], in_=agout[0:32, 0:32])
                nc.scalar.dma_start(out=gpre[32:64, 0:32], in_=agout[0:32, 32:64])
                nc.vector.transpose(gpT[:], gpre[:])
                gall = gpT[:, 0:32].rearrange("b (r s) -> b r s", s=2 * NCAND)
                gv = gall[:, :, 0:NCAND]
                gi = gall[:, :, NCAND : 2 * NCAND]

                # ---- global argmax over 16 exact candidates (min-id tiebreak) ----
                nc.vector.max(out=gm8[:], in_=gv)
                nc.vector.tensor_scalar(msk[:], gv, gm8[:, 0:1], BIG, op0=OP.is_lt, op1=OP.mult)
                nc.vector.tensor_tensor(msk[:], gi, msk[:], op=OP.add)
                nc.vector.tensor_reduce(gidxf[:], msk[:], axis=AX.XY, op=OP.min)
                nc.vector.tensor_copy(gidx[:], gidxf[:])

                # ---- gather the token's precomputed gate row in two halves:
                # [f,g] lands first so the pointwise adds start early ----
                nc.gpsimd.indirect_dma_start(
                    out=gx[:, 0:HG], out_offset=None, in_=whe0_p[:],
                    in_offset=bass.IndirectOffsetOnAxis(ap=gidx[:, :1], axis=0),
                )
                nc.gpsimd.indirect_dma_start(
                    out=gx[:, HG:G4], out_offset=None, in_=whe1_p[:],
                    in_offset=bass.IndirectOffsetOnAxis(ap=gidx[:, :1], axis=0),
                )

    nc.finalize()  # Bacc: runs compile() legalization passes
    return nc


# gate-unit permutation [f, g, i, o] (torch order in the weights is i, f, g, o)
_PERM = np.concatenate([np.arange(512, 1024), np.arange(1024, 1536),
                        np.arange(0, 512), np.arange(1536, 2048)])


def make_in_maps(inputs):
    inp = {k: np.asarray(v) for k, v in inputs.items()}
    h0 = inp["h0"].astype(np.float32)
    c0 = inp["c0"].astype(np.float32)
    W_ih = inp["W_ih"].astype(np.float32)
    W_hh = inp["W_hh"].astype(np.float32)
    b = (inp["b_ih"].astype(np.float32) + inp["b_hh"].astype(np.float32))
    W_out = inp["W_out"].astype(np.float32)
    b_out = inp["b_out"].astype(np.float32)
    emb = inp["embed_table"].astype(np.float32)
    # x @ W_ih.T + b for every vocab row, fp32, gate units permuted to [f,g,i,o]
    whe = np.ascontiguousarray((emb @ W_ih.T + b)[:, _PERM].astype(np.float32))
    whe0 = np.ascontiguousarray(whe[:, 0 : G4 // 2])
    whe1 = np.ascontiguousarray(whe[:, G4 // 2 : G4])
    wfullb = np.ascontiguousarray(
        np.concatenate([W_out, b_out.reshape(V, 1)], axis=1).astype(np.float32))
    whhT = np.ascontiguousarray(W_hh[_PERM].T)
    bias = np.ascontiguousarray(b[_PERM].reshape(1, G4))
    in_maps = []
    for c in range(NCORES):
        base = c * VS
        tbase = np.zeros((B, NT * TPT), np.float32)
        for n in range(NT):
            tbase[:, TPT * n : TPT * (n + 1)] = float(base + n * TN)
        in_maps.append({
            "h0T": np.ascontiguousarray(h0.T),
            "c0": np.ascontiguousarray(c0),
            "whhT": whhT,
            "bias": bias,
            "woutT": np.ascontiguousarray(W_out[base : base + VS].T),
            "bout": np.ascontiguousarray(b_out[base : base + VS].reshape(1, VS)),
            "whe0": whe0,
            "whe1": whe1,
            "wfullb": wfullb,
            "tbase": tbase,
        })
    return in_maps


def run(inputs, steps=S, trace=False):
    from concourse.bass_utils import run_bass_kernel_spmd

    nc = build_program(steps)
    res = run_bass_kernel_spmd(nc, make_in_maps(inputs), list(range(NCORES)),
                               trace=trace)
    outs = [np.asarray(res.results[c]["out"]).astype(np.float32)
            for c in range(NCORES)]                            # each [steps, B, VS]
    full = np.concatenate(outs, axis=2)                        # [steps, B, V]
    return np.ascontiguousarray(np.transpose(full, (1, 0, 2))), res


def kernel(**inputs):
    out, _ = run(inputs, steps=S, trace=False)
    return out.astype(np.float32)


# revision 30
# speedup vs baseline: 1.0018x; 1.0018x over previous
"""LSTM greedy decoder on 8 trn2 NeuronCores.

Vocab-parallel: each core keeps a resident fp32r SBUF copy of its
4000-row W_out shard, replicates the LSTM cell (exact fp32), and agrees
on the greedy token by exchanging per-core top-2 exact candidates via a
tiny AllGather.

Scheduling notes (r1):
- per-tile MAX8/MAX_INDEX8 run pipelined under the logits matmuls and
  write straight into persistent [B, NT, 8] slots (no per-tile copies).
- the b_out bias-add runs on GpSimd reading the PSUM tile directly and
  writing the SBUF logits tile (the scalar PSUM->SBUF copy is gone).
- the top-2 candidate logits are re-evaluated exactly in fp32 BEFORE
  the AllGather; each candidate's W_out row gather is issued as soon as
  its id is known (overlapping the next selection round), and each dot
  is one fused tensor_tensor_reduce.
- AllGather staging uses DVE 32x32 stream-transposes + contiguous DMAs.
- the whe token-row gather is split in two column halves ([f,g] then
  [i,o]) so the LSTM pointwise can start on the first half early.
- LSTM pointwise is choreographed across engines: per-gate adds on
  Vector (+o on GpSimd), activations on Scalar in dependency order
  (sigmoid f, tanh g, sigmoid i, sigmoid o), fc on GpSimd.
- gate layout is host-permuted to [f, g, i, o].
- gates matmuls for step t+1 are emitted after the logits so the PE
  works under the collective tail.
- dense PE warm bursts (dummy transposes reading constants, so no data
  deps) keep the PE p-state high across the pointwise window.
- all single-buffer tail scratch tiles are allocated once, outside the
  step loop.
"""

import numpy as np

B, H, D, V, S = 64, 512, 256, 32000, 64
NCORES = 8
VS = V // NCORES            # 4000 vocab rows per core
G4 = 4 * H                  # 2048 gate units
NT = 8                      # logits N-tiles per step
TN = VS // NT               # 500 columns per logits tile
KH = H // 128               # 4 contraction tiles over H
BIG = 1.0e9
NCAND = 2                   # exact-rechecked candidates per core
TPT = 3                     # per-tile candidates pooled


def build_program(steps=S):
    import concourse.bass as bass
    import concourse.bacc as bacc
    import concourse.mybir as mybir
    import concourse.tile as tile
    from concourse.masks import make_identity

    f32 = mybir.dt.float32
    bf16 = mybir.dt.bfloat16
    u32 = mybir.dt.uint32
    AF = mybir.ActivationFunctionType
    OP = mybir.AluOpType
    AX = mybir.AxisListType

    nc = bacc.Bacc(num_devices=NCORES)
    h0T_p = nc.declare_dram_parameter("h0T", [H, B], f32, isOutput=False)
    c0_p = nc.declare_dram_parameter("c0", [B, H], f32, isOutput=False)
    whhT_p = nc.declare_dram_parameter("whhT", [H, G4], f32, isOutput=False)
    bias_p = nc.declare_dram_parameter("bias", [1, G4], f32, isOutput=False)
    woutT_p = nc.declare_dram_parameter("woutT", [H, VS], f32, isOutput=False)
    bout_p = nc.declare_dram_parameter("bout", [1, VS], f32, isOutput=False)
    whe0_p = nc.declare_dram_parameter("whe0", [V, G4 // 2], f32, isOutput=False)
    whe1_p = nc.declare_dram_parameter("whe1", [V, G4 // 2], f32, isOutput=False)
    wfullb_p = nc.declare_dram_parameter("wfullb", [V, H + 1], f32, isOutput=False)
    tbase_p = nc.declare_dram_parameter("tbase", [B, NT * TPT], f32, isOutput=False)
    out_p = nc.declare_dram_parameter("out", [steps, B, VS], bf16, isOutput=True)

    rg = [list(range(NCORES))]
    HG = G4 // 2  # 1024: [f,g] | [i,o] halves
    sF = slice(0, 512)
    sG = slice(512, 1024)
    sI = slice(1024, 1536)
    sO = slice(1536, 2048)

    with tile.TileContext(nc) as tc:
        with (
            tc.tile_pool(name="wpool", bufs=1) as wp,
            tc.tile_pool(name="state", bufs=2) as sp,
            tc.tile_pool(name="work", bufs=2) as kp,
            tc.tile_pool(name="ps_g", bufs=1, space="PSUM") as pg,
            tc.tile_pool(name="ps_l", bufs=2, space="PSUM") as pl,
            tc.tile_pool(name="ps_t", bufs=2, space="PSUM") as pt,
            tc.tile_pool(name="dram", bufs=2, space="DRAM") as dp,
        ):
            # ---- constants (engine-local, no DMA) ----
            ident = wp.tile([128, 128], f32)
            make_identity(nc, ident[:])
            ones1 = wp.tile([1, B], f32)
            nc.vector.memset(ones1[:], 1.0)

            # ---- resident weights (barriers cap per-inst sync-wait fan-in) ----
            tc.strict_bb_all_engine_barrier()
            whh = wp.tile([128, KH, G4], f32)
            nc.sync.dma_start(out=whh[:], in_=whhT_p[:].rearrange("(a p) n -> p a n", p=128))
            bias = wp.tile([1, G4], f32)
            nc.sync.dma_start(out=bias[:], in_=bias_p[:])
            tbase = wp.tile([B, NT * TPT], f32)
            nc.sync.dma_start(out=tbase[:], in_=tbase_p[:])
            tc.strict_bb_all_engine_barrier()

            # bf16 logits weights: stage fp32 chunks through the logits-tagged
            # slot, then round-copy. bf16 matmul runs 1 cyc/row (vs fp32r's
            # effective 2) and at ~half the PE power, which also relieves the
            # HAM util-limit throttle windows. The exact fp32 recheck of the
            # top-2 candidates keeps the token feedback path exact.
            wout = wp.tile([128, KH, VS], bf16)
            wq = woutT_p[:].rearrange("(a p) n -> p a n", p=128)
            for k in range(KH):
                stage = kp.tile([128, VS], f32, tag="logits", name=f"wstage{k}")
                nc.sync.dma_start(out=stage[:], in_=wq[:, k, :])
                nc.vector.tensor_copy(wout[:, k, :], stage[:])
                tc.strict_bb_all_engine_barrier()
            # broadcast b_out across the 64 batch partitions once
            biasb = wp.tile([B, VS], f32)
            bstage = kp.tile([1, VS], f32, tag="logits", name="bstage")
            nc.sync.dma_start(out=bstage[:], in_=bout_p[:])
            for n in range(NT):
                ns = slice(n * TN, (n + 1) * TN)
                lg = pl.tile([B, TN], f32, tag="lg")
                nc.tensor.matmul(out=lg[:], lhsT=ones1[:], rhs=bstage[:, ns],
                                 start=True, stop=True)
                nc.scalar.activation(biasb[:, ns], lg[:], AF.Copy)
            tc.strict_bb_all_engine_barrier()

            # ---- loop-invariant scratch (alloc once: per-iter bufs=1
            # re-allocs hit the min-join fallback and stall on Tensor) ----
            pk = wp.tile([B, 32], f32)          # [ev0 ev1 | id0 id1 | pad]
            nc.vector.memset(pk[:], 0.0)
            pkT = wp.tile([B, 32], f32)
            gpre = wp.tile([B, 32], f32)
            nc.vector.memset(gpre[:], 0.0)
            gpT = wp.tile([B, 32], f32)
            m8all = wp.tile([B, NT, 8], f32)    # per-tile sorted top-8 values
            i8all = wp.tile([B, NT, 8], u32)    # per-tile top-8 indices
            tif = wp.tile([B, NT, TPT], f32)    # pooled candidate global ids
            m8l = wp.tile([B, 8], f32)
            si = wp.tile([B, NCAND], f32)
            dcand = wp.tile([B, NT, TPT], f32)
            dhi = wp.tile([B, NT, TPT], f32)
            cu = wp.tile([B, NCAND], u32)
            wrow3 = wp.tile([B, NCAND, H + 1], f32)
            prod3 = wp.tile([B, NCAND, H + 1], f32)
            gm8 = wp.tile([B, 8], f32)
            msk = wp.tile([B, NCORES, NCAND], f32)
            gidxf = wp.tile([B, 1], f32)
            gidx = wp.tile([B, 1], u32)
            gx = wp.tile([B, G4], f32)
            nc.vector.memset(gx[:], 0.0)
            gsb = wp.tile([B, G4], f32)
            acts = wp.tile([B, G4], f32)
            fc = wp.tile([B, H], f32)
            ig = wp.tile([B, H], f32)
            tct = wp.tile([B, H], f32)
            h1a = wp.tile([B, H + 1], f32)
            nc.vector.memset(h1a[:, H : H + 1], 1.0)
            h1 = h1a[:, 0:H]

            hT = sp.tile([128, KH, B], f32, tag="hT")
            nc.sync.dma_start(out=hT[:], in_=h0T_p[:].rearrange("(a p) b -> p a b", p=128))
            tc.strict_bb_all_engine_barrier()
            hTr = sp.tile([128, KH, B], bf16, tag="hTr")
            nc.vector.tensor_copy(hTr[:], hT[:])
            c_prev = sp.tile([B, H], f32, tag="c")
            nc.sync.dma_start(out=c_prev[:], in_=c0_p[:])
            tc.strict_bb_all_engine_barrier()

            # ---- gates for t=0: bias + W_hh @ h0 (exact fp32) ----
            gates = pg.tile([B, G4], f32, tag="gates")
            for n in range(4):
                ns = slice(n * 512, (n + 1) * 512)
                nc.tensor.matmul(out=gates[:, ns], lhsT=ones1[:], rhs=bias[:, ns],
                                 start=True, stop=False)
                for k in range(KH):
                    nc.tensor.matmul(out=gates[:, ns], lhsT=hT[:, k, :], rhs=whh[:, k, ns],
                                     start=False, stop=(k == KH - 1))

            first = True
            for t in range(steps):
                # ---- LSTM pointwise; gate layout is [f, g, i, o] ----
                # per-gate source: t=0 reads gates PSUM directly (no x-part);
                # later steps add the gathered x-contribution first.
                if first:
                    src = gates
                else:
                    nc.vector.tensor_tensor(gsb[:, sF], gates[:, sF], gx[:, sF], op=OP.add)
                    nc.vector.tensor_tensor(gsb[:, sG], gates[:, sG], gx[:, sG], op=OP.add)
                    nc.vector.tensor_tensor(gsb[:, sI], gates[:, sI], gx[:, sI], op=OP.add)
                    nc.vector.tensor_tensor(gsb[:, sO], gates[:, sO], gx[:, sO], op=OP.add)
                    src = gsb
                nc.scalar.activation(acts[:, sF], src[:, sF], AF.Sigmoid)
                nc.scalar.activation(acts[:, sG], src[:, sG], AF.Tanh)
                nc.scalar.activation(acts[:, sI], src[:, sI], AF.Sigmoid)
                nc.scalar.activation(acts[:, sO], src[:, sO], AF.Sigmoid)
                first = False
                # dense PE warm burst with deps staggered through the
                # pointwise window (gx -> acts -> tct) so the PE stays near
                # its high p-state right up to the hT transposes
                for w in range(4):
                    wmt = pt.tile([128, B], f32, tag="tp")
                    nc.tensor.transpose(out=wmt[:], in_=gx[:, 128 * w : 128 * (w + 1)],
                                        identity=ident[0:B, 0:B])
                for src_ap in (acts[:, 0:128], acts[:, 512:640], acts[:, 1024:1152],
                               acts[:, 1536:1664]):
                    wmt = pt.tile([128, B], f32, tag="tp")
                    nc.tensor.transpose(out=wmt[:], in_=src_ap, identity=ident[0:B, 0:B])
                nc.gpsimd.tensor_tensor(fc[:], acts[:, sF], c_prev[:], op=OP.mult)
                nc.vector.tensor_tensor(ig[:], acts[:, sI], acts[:, sG], op=OP.mult)
                c_new = sp.tile([B, H], f32, tag="c")
                nc.vector.tensor_tensor(c_new[:], fc[:], ig[:], op=OP.add)
                nc.scalar.activation(tct[:], c_new[:], AF.Tanh)
                for w in range(2):
                    wmt = pt.tile([128, B], f32, tag="tp")
                    nc.tensor.transpose(out=wmt[:], in_=tct[:, 128 * w : 128 * (w + 1)],
                                        identity=ident[0:B, 0:B])
                nc.vector.tensor_tensor(h1[:], acts[:, sO], tct[:], op=OP.mult)
                c_prev = c_new

                # ---- h1 -> h1.T tiles (PE transpose); both copies on DVE so
                # the logits (hTr) unblock no later than the gates (hT) ----
                hT = sp.tile([128, KH, B], f32, tag="hT")
                hTr = sp.tile([128, KH, B], bf16, tag="hTr")
                for k in range(KH):
                    tp = pt.tile([128, B], f32, tag="tp")
                    nc.tensor.transpose(out=tp[:], in_=h1[:, k * 128 : (k + 1) * 128],
                                        identity=ident[0:B, 0:B])
                    nc.vector.tensor_copy(hTr[:, k, :], tp[:])
                    nc.vector.tensor_copy(hT[:, k, :], tp[:])

                # ---- logits shard in bf16 + per-tile top-8 scan ----
                # scalar drains PSUM -> SBUF, GpSimd adds b_out in place,
                # MAX8/MAX_INDEX8 on the biased f32 SBUF tile (exact f32
                # values, so no duplicate-tie hazard); a second scalar copy
                # mirrors each tile to bf16 so the output write is half-size
                logits = kp.tile([B, VS], f32, tag="logits")
                outb = kp.tile([B, VS], bf16, tag="outb")
                last = t == steps - 1
                for n in range(NT):
                    ns = slice(n * TN, (n + 1) * TN)
                    lg = pl.tile([B, TN], f32, tag="lg")
                    for k in range(KH):
                        nc.tensor.matmul(out=lg[:], lhsT=hTr[:, k, :], rhs=wout[:, k, ns],
                                         start=(k == 0), stop=(k == KH - 1))
                    nc.scalar.activation(logits[:, ns], lg[:], AF.Copy)
                    nc.gpsimd.tensor_tensor(logits[:, ns], logits[:, ns], biasb[:, ns], op=OP.add)
                    if not last:
                        nc.vector.max(out=m8all[:, n, :], in_=logits[:, ns])
                        nc.vector.max_index(out=i8all[:, n, :], in_max=m8all[:, n, :],
                                            in_values=logits[:, ns])
                # bf16 mirrors emitted after the scan loop so they backfill
                # scalar idle time instead of ping-ponging with the GpSimd adds
                for n in range(NT):
                    ns = slice(n * TN, (n + 1) * TN)
                    nc.scalar.activation(outb[:, ns], logits[:, ns], AF.Copy)
                if last:
                    nc.scalar.dma_start(out=out_p[t], in_=outb[:])
                    break

                # ---- gates h-part for t+1 (exact fp32) — PE works under the tail ----
                gates = pg.tile([B, G4], f32, tag="gates")
                for n in range(4):
                    ns = slice(n * 512, (n + 1) * 512)
                    for k in range(KH):
                        nc.tensor.matmul(out=gates[:, ns], lhsT=hT[:, k, :], rhs=whh[:, k, ns],
                                         start=(k == 0), stop=(k == KH - 1))

                # ---- local top-2 (distinct ids) from the 24 pooled candidates;
                # each candidate's W_out row gather is issued as soon as its id
                # is known so the transfer overlaps the next selection round ----
                # top-2 by the sorted pooled values: slot k keeps entries in
                # the value band [m8l[k], m8l[k-1]) via (v is_lt m8l[k])*BIG +
                # (v is_ge m8l[k-1])*BIG + id, then min-reduce. Both chains
                # depend only on m8l, so the candidate-0 row gather issues
                # while the candidate-1 selection still runs. (Exact f32 value
                # ties across distinct ids are ~impossible.)
                tv = m8all[:, :, 0:TPT]
                nc.vector.tensor_copy(tif[:], i8all[:, :, 0:TPT])
                nc.vector.tensor_tensor(tif[:], tif[:], tbase[:].rearrange("b (n c) -> b n c", c=TPT), op=OP.add)
                nc.vector.max(out=m8l[:], in_=tv)
                for k in range(NCAND):
                    nc.vector.tensor_scalar(dcand[:], tv, m8l[:, k : k + 1], BIG, op0=OP.is_lt, op1=OP.mult)
                    if k > 0:
                        nc.vector.tensor_scalar(dhi[:], tv, m8l[:, k - 1 : k], BIG, op0=OP.is_ge, op1=OP.mult)
                        nc.vector.tensor_tensor(dcand[:], dcand[:], dhi[:], op=OP.add)
                    nc.vector.tensor_tensor(dcand[:], tif[:], dcand[:], op=OP.add)
                    nc.vector.tensor_reduce(si[:, k : k + 1], dcand[:], axis=AX.XY, op=OP.min)
                    nc.vector.tensor_copy(cu[:, k : k + 1], si[:, k : k + 1])
                    nc.gpsimd.indirect_dma_start(
                        out=wrow3[:, k], out_offset=None, in_=wfullb_p[:],
                        in_offset=bass.IndirectOffsetOnAxis(ap=cu[:, k : k + 1], axis=0),
                    )
                nc.vector.tensor_copy(pk[:, NCAND : 2 * NCAND], si[:])

                # ---- exact fp32 recheck of the candidates ----
                for j in range(NCAND):
                    nc.vector.tensor_tensor(prod3[:, j], wrow3[:, j], h1a[:], op=OP.mult)
                nc.vector.tensor_reduce(pk[:, 0:NCAND], prod3[:], axis=AX.X, op=OP.add)

                # ---- AllGather of (2 exact vals | 2 ids) per core ----
                nc.vector.transpose(pkT[:], pk[:])
                agin = dp.tile([2 * NCAND, B], f32, tag="agin")
                nc.sync.dma_start(out=agin[:, 0:32], in_=pkT[0 : 2 * NCAND, 0:32])
                nc.scalar.dma_start(out=agin[:, 32:64], in_=pkT[32 : 32 + 2 * NCAND, 0:32])
                # the 0.5MB bf16 logits write is issued here so most of its
                # transfers ride inside the collective window instead of
                # contending with the recheck row gathers
                nc.sync.dma_start(out=out_p[t], in_=outb[:])
                agout = dp.tile([NCORES * 2 * NCAND, B], f32, tag="agout", addr_space="Shared")
                nc.gpsimd.collective_compute(
                    "AllGather", OP.bypass, replica_groups=rg,
                    ins=[agin[:].opt()], outs=[agout[:].opt()],
                )
                # block-swapped readback so one DVE stream-transpose finishes it
                nc.sync.dma_start(out=gpre[0:32, 0:32], in_=agout[0:32, 0:32])
                nc.scalar.dma_start(out=gpre[32:64, 0:32], in_=agout[0:32, 32:64])
                nc.vector.transpose(gpT[:], gpre[:])
                gall = gpT[:, 0:32].rearrange("b (r s) -> b r s", s=2 * NCAND)
                gv = gall[:, :, 0:NCAND]
                gi = gall[:, :, NCAND : 2 * NCAND]

                # ---- global argmax over 16 exact candidates (min-id tiebreak) ----
                nc.vector.max(out=gm8[:], in_=gv)
                nc.vector.tensor_scalar(msk[:], gv, gm8[:, 0:1], BIG, op0=OP.is_lt, op1=OP.mult)
                nc.vector.tensor_tensor(msk[:], gi, msk[:], op=OP.add)
                nc.vector.tensor_reduce(gidxf[:], msk[:], axis=AX.XY, op=OP.min)
                nc.vector.tensor_copy(gidx[:], gidxf[:])

                # ---- gather the token's precomputed gate row in two halves:
                # [f,g] lands first so the pointwise adds start early ----
                nc.gpsimd.indirect_dma_start(
                    out=gx[:, 0:HG], out_offset=None, in_=whe0_p[:],
                    in_offset=bass.IndirectOffsetOnAxis(ap=gidx[:, :1], axis=0),
                )
                nc.gpsimd.indirect_dma_start(
                    out=gx[:, HG:G4], out_offset=None, in_=whe1_p[:],
                    in_offset=bass.IndirectOffsetOnAxis(ap=gidx[:, :1], axis=0),
                )

    nc.finalize()  # Bacc: runs compile() legalization passes
    return nc


# gate-unit permutation [f, g, i, o] (torch order in the weights is i, f, g, o)
_PERM = np.concatenate([np.arange(512, 1024), np.arange(1024, 1536),
                        np.arange(0, 512), np.arange(1536, 2048)])


def make_in_maps(inputs):
    inp = {k: np.asarray(v) for k, v in inputs.items()}
    h0 = inp["h0"].astype(np.float32)
    c0 = inp["c0"].astype(np.float32)
    W_ih = inp["W_ih"].astype(np.float32)
    W_hh = inp["W_hh"].astype(np.float32)
    b = (inp["b_ih"].astype(np.float32) + inp["b_hh"].astype(np.float32))
    W_out = inp["W_out"].astype(np.float32)
    b_out = inp["b_out"].astype(np.float32)
    emb = inp["embed_table"].astype(np.float32)
    # x @ W_ih.T + b for every vocab row, fp32, gate units permuted to [f,g,i,o]
    whe = np.ascontiguousarray((emb @ W_ih.T + b)[:, _PERM].astype(np.float32))
    whe0 = np.ascontiguousarray(whe[:, 0 : G4 // 2])
    whe1 = np.ascontiguousarray(whe[:, G4 // 2 : G4])
    wfullb = np.ascontiguousarray(
        np.concatenate([W_out, b_out.reshape(V, 1)], axis=1).astype(np.float32))
    whhT = np.ascontiguousarray(W_hh[_PERM].T)
    bias = np.ascontiguousarray(b[_PERM].reshape(1, G4))
    in_maps = []
    for c in range(NCORES):
        base = c * VS
        tbase = np.zeros((B, NT * TPT), np.float32)
        for n in range(NT):
            tbase[:, TPT * n : TPT * (n + 1)] = float(base + n * TN)
        in_maps.append({
            "h0T": np.ascontiguousarray(h0.T),
            "c0": np.ascontiguousarray(c0),
            "whhT": whhT,
            "bias": bias,
            "woutT": np.ascontiguousarray(W_out[base : base + VS].T),
            "bout": np.ascontiguousarray(b_out[base : base + VS].reshape(1, VS)),
            "whe0": whe0,
            "whe1": whe1,
            "wfullb": wfullb,
            "tbase": tbase,
        })
    return in_maps


def run(inputs, steps=S, trace=False):
    from concourse.bass_utils import run_bass_kernel_spmd

    nc = build_program(steps)
    res = run_bass_kernel_spmd(nc, make_in_maps(inputs), list(range(NCORES)),
                               trace=trace)
    outs = [np.asarray(res.results[c]["out"]).astype(np.float32)
            for c in range(NCORES)]                            # each [steps, B, VS]
    full = np.concatenate(outs, axis=2)                        # [steps, B, V]
    return np.ascontiguousarray(np.transpose(full, (1, 0, 2))), res


def kernel(**inputs):
    out, _ = run(inputs, steps=S, trace=False)
    return out.astype(np.float32)


# revision 35
# speedup vs baseline: 1.0653x; 1.0633x over previous
"""LSTM greedy decoder on 8 trn2 NeuronCores.

Vocab-parallel: each core keeps a resident fp32r SBUF copy of its
4000-row W_out shard, replicates the LSTM cell (exact fp32), and agrees
on the greedy token by exchanging per-core top-2 exact candidates via a
tiny AllGather.

Scheduling notes (r1):
- per-tile MAX8/MAX_INDEX8 run pipelined under the logits matmuls and
  write straight into persistent [B, NT, 8] slots (no per-tile copies).
- the b_out bias-add runs on GpSimd reading the PSUM tile directly and
  writing the SBUF logits tile (the scalar PSUM->SBUF copy is gone).
- the top-2 candidate logits are re-evaluated exactly in fp32 BEFORE
  the AllGather; each candidate's W_out row gather is issued as soon as
  its id is known (overlapping the next selection round), and each dot
  is one fused tensor_tensor_reduce.
- AllGather staging uses DVE 32x32 stream-transposes + contiguous DMAs.
- the whe token-row gather is split in two column halves ([f,g] then
  [i,o]) so the LSTM pointwise can start on the first half early.
- LSTM pointwise is choreographed across engines: per-gate adds on
  Vector (+o on GpSimd), activations on Scalar in dependency order
  (sigmoid f, tanh g, sigmoid i, sigmoid o), fc on GpSimd.
- gate layout is host-permuted to [f, g, i, o].
- gates matmuls for step t+1 are emitted after the logits so the PE
  works under the collective tail.
- dense PE warm bursts (dummy transposes reading constants, so no data
  deps) keep the PE p-state high across the pointwise window.
- all single-buffer tail scratch tiles are allocated once, outside the
  step loop.
"""

import numpy as np

B, H, D, V, S = 64, 512, 256, 32000, 64
NCORES = 8
VS = V // NCORES            # 4000 vocab rows per core
G4 = 4 * H                  # 2048 gate units
NT = 8                      # logits N-tiles per step
TN = VS // NT               # 500 columns per logits tile
KH = H // 128               # 4 contraction tiles over H
BIG = 1.0e9
NCAND = 2                   # exact-rechecked candidates per core
TPT = 3                     # per-tile candidates pooled


def build_program(steps=S):
    import concourse.bass as bass
    import concourse.bacc as bacc
    import concourse.mybir as mybir
    import concourse.tile as tile
    from concourse.masks import make_identity

    f32 = mybir.dt.float32
    bf16 = mybir.dt.bfloat16
    u32 = mybir.dt.uint32
    AF = mybir.ActivationFunctionType
    OP = mybir.AluOpType
    AX = mybir.AxisListType

    nc = bacc.Bacc(num_devices=NCORES)
    h0T_p = nc.declare_dram_parameter("h0T", [H, B], f32, isOutput=False)
    c0_p = nc.declare_dram_parameter("c0", [B, H], f32, isOutput=False)
    whhT_p = nc.declare_dram_parameter("whhT", [H, G4], f32, isOutput=False)
    bias_p = nc.declare_dram_parameter("bias", [1, G4], f32, isOutput=False)
    woutT_p = nc.declare_dram_parameter("woutT", [H, VS], f32, isOutput=False)
    bout_p = nc.declare_dram_parameter("bout", [1, VS], f32, isOutput=False)
    whe0_p = nc.declare_dram_parameter("whe0", [V, G4 // 2], f32, isOutput=False)
    whe1_p = nc.declare_dram_parameter("whe1", [V, G4 // 2], f32, isOutput=False)
    wfullb_p = nc.declare_dram_parameter("wfullb", [V, H + 1], f32, isOutput=False)
    tbase_p = nc.declare_dram_parameter("tbase", [B, NT * TPT], f32, isOutput=False)
    out_p = nc.declare_dram_parameter("out", [steps, B, VS], f32, isOutput=True)

    rg = [list(range(NCORES))]
    HG = G4 // 2  # 1024: [f,g] | [i,o] halves
    sF = slice(0, 512)
    sG = slice(512, 1024)
    sI = slice(1024, 1536)
    sO = slice(1536, 2048)

    with tile.TileContext(nc) as tc:
        with (
            tc.tile_pool(name="wpool", bufs=1) as wp,
            tc.tile_pool(name="state", bufs=2) as sp,
            tc.tile_pool(name="work", bufs=2) as kp,
            tc.tile_pool(name="ps_g", bufs=1, space="PSUM") as pg,
            tc.tile_pool(name="ps_l", bufs=2, space="PSUM") as pl,
            tc.tile_pool(name="ps_t", bufs=2, space="PSUM") as pt,
            tc.tile_pool(name="dram", bufs=2, space="DRAM") as dp,
        ):
            # ---- constants (engine-local, no DMA) ----
            ident = wp.tile([128, 128], f32)
            make_identity(nc, ident[:])
            ones1 = wp.tile([1, B], f32)
            nc.vector.memset(ones1[:], 1.0)

            # ---- resident weights (barriers cap per-inst sync-wait fan-in) ----
            tc.strict_bb_all_engine_barrier()
            whh = wp.tile([128, KH, G4], f32)
            nc.sync.dma_start(out=whh[:], in_=whhT_p[:].rearrange("(a p) n -> p a n", p=128))
            bias = wp.tile([1, G4], f32)
            nc.sync.dma_start(out=bias[:], in_=bias_p[:])
            tbase = wp.tile([B, NT * TPT], f32)
            nc.sync.dma_start(out=tbase[:], in_=tbase_p[:])
            tc.strict_bb_all_engine_barrier()

            # bf16 logits weights: stage fp32 chunks through the logits-tagged
            # slot, then round-copy. bf16 matmul runs 1 cyc/row (vs fp32r's
            # effective 2) and at ~half the PE power, which also relieves the
            # HAM util-limit throttle windows. The exact fp32 recheck of the
            # top-2 candidates keeps the token feedback path exact.
            wout = wp.tile([128, KH, VS], bf16)
            wq = woutT_p[:].rearrange("(a p) n -> p a n", p=128)
            for k in range(KH):
                stage = kp.tile([128, VS], f32, tag="logits", name=f"wstage{k}")
                nc.sync.dma_start(out=stage[:], in_=wq[:, k, :])
                nc.vector.tensor_copy(wout[:, k, :], stage[:])
                tc.strict_bb_all_engine_barrier()
            # broadcast b_out across the 64 batch partitions once
            biasb = wp.tile([B, VS], f32)
            bstage = kp.tile([1, VS], f32, tag="logits", name="bstage")
            nc.sync.dma_start(out=bstage[:], in_=bout_p[:])
            for n in range(NT):
                ns = slice(n * TN, (n + 1) * TN)
                lg = pl.tile([B, TN], f32, tag="lg")
                nc.tensor.matmul(out=lg[:], lhsT=ones1[:], rhs=bstage[:, ns],
                                 start=True, stop=True)
                nc.scalar.activation(biasb[:, ns], lg[:], AF.Copy)
            tc.strict_bb_all_engine_barrier()

            # ---- loop-invariant scratch (alloc once: per-iter bufs=1
            # re-allocs hit the min-join fallback and stall on Tensor) ----
            pk = wp.tile([B, 32], f32)          # [ev0 ev1 | id0 id1 | pad]
            nc.vector.memset(pk[:], 0.0)
            pkT = wp.tile([B, 32], f32)
            gpre = wp.tile([B, 32], f32)
            nc.vector.memset(gpre[:], 0.0)
            gpT = wp.tile([B, 32], f32)
            m8all = wp.tile([B, NT, 8], f32)    # per-tile sorted top-8 values
            i8all = wp.tile([B, NT, 8], u32)    # per-tile top-8 indices
            tif = wp.tile([B, NT, TPT], f32)    # pooled candidate global ids
            m8l = wp.tile([B, 8], f32)
            si = wp.tile([B, NCAND], f32)
            dcand = wp.tile([B, NT, TPT], f32)
            dhi = wp.tile([B, NT, TPT], f32)
            cu = wp.tile([B, NCAND], u32)
            wrow3 = wp.tile([B, NCAND, H + 1], f32)
            prod3 = wp.tile([B, NCAND, H + 1], f32)
            gm8 = wp.tile([B, 8], f32)
            msk = wp.tile([B, NCORES, NCAND], f32)
            gidxf = wp.tile([B, 1], f32)
            gidx = wp.tile([B, 1], u32)
            gx = wp.tile([B, G4], f32)
            nc.vector.memset(gx[:], 0.0)
            gsb = wp.tile([B, G4], f32)
            acts = wp.tile([B, G4], f32)
            fc = wp.tile([B, H], f32)
            ig = wp.tile([B, H], f32)
            tct = wp.tile([B, H], f32)
            h1a = wp.tile([B, H + 1], f32)
            nc.vector.memset(h1a[:, H : H + 1], 1.0)
            h1 = h1a[:, 0:H]

            hT = sp.tile([128, KH, B], f32, tag="hT")
            nc.sync.dma_start(out=hT[:], in_=h0T_p[:].rearrange("(a p) b -> p a b", p=128))
            tc.strict_bb_all_engine_barrier()
            hTr = sp.tile([128, KH, B], bf16, tag="hTr")
            nc.vector.tensor_copy(hTr[:], hT[:])
            c_prev = sp.tile([B, H], f32, tag="c")
            nc.sync.dma_start(out=c_prev[:], in_=c0_p[:])
            tc.strict_bb_all_engine_barrier()

            # ---- gates for t=0: bias + W_hh @ h0 (exact fp32) ----
            gates = pg.tile([B, G4], f32, tag="gates")
            for n in range(4):
                ns = slice(n * 512, (n + 1) * 512)
                nc.tensor.matmul(out=gates[:, ns], lhsT=ones1[:], rhs=bias[:, ns],
                                 start=True, stop=False)
                for k in range(KH):
                    nc.tensor.matmul(out=gates[:, ns], lhsT=hT[:, k, :], rhs=whh[:, k, ns],
                                     start=False, stop=(k == KH - 1))

            first = True
            for t in range(steps):
                # ---- LSTM pointwise; gate layout is [f, g, i, o] ----
                # per-gate source: t=0 reads gates PSUM directly (no x-part);
                # later steps add the gathered x-contribution first.
                if first:
                    src = gates
                else:
                    nc.vector.tensor_tensor(gsb[:, sF], gates[:, sF], gx[:, sF], op=OP.add)
                    nc.vector.tensor_tensor(gsb[:, sG], gates[:, sG], gx[:, sG], op=OP.add)
                    nc.vector.tensor_tensor(gsb[:, sI], gates[:, sI], gx[:, sI], op=OP.add)
                    nc.vector.tensor_tensor(gsb[:, sO], gates[:, sO], gx[:, sO], op=OP.add)
                    src = gsb
                nc.scalar.activation(acts[:, sF], src[:, sF], AF.Sigmoid)
                nc.scalar.activation(acts[:, sG], src[:, sG], AF.Tanh)
                nc.scalar.activation(acts[:, sI], src[:, sI], AF.Sigmoid)
                nc.scalar.activation(acts[:, sO], src[:, sO], AF.Sigmoid)
                first = False
                # dense PE warm burst with deps staggered through the
                # pointwise window (gx -> acts -> tct) so the PE stays near
                # its high p-state right up to the hT transposes
                for w in range(4):
                    wmt = pt.tile([128, B], f32, tag="tp")
                    nc.tensor.transpose(out=wmt[:], in_=gx[:, 128 * w : 128 * (w + 1)],
                                        identity=ident[0:B, 0:B])
                for src_ap in (acts[:, 0:128], acts[:, 512:640], acts[:, 1024:1152],
                               acts[:, 1536:1664]):
                    wmt = pt.tile([128, B], f32, tag="tp")
                    nc.tensor.transpose(out=wmt[:], in_=src_ap, identity=ident[0:B, 0:B])
                nc.gpsimd.tensor_tensor(fc[:], acts[:, sF], c_prev[:], op=OP.mult)
                nc.vector.tensor_tensor(ig[:], acts[:, sI], acts[:, sG], op=OP.mult)
                c_new = sp.tile([B, H], f32, tag="c")
                nc.vector.tensor_tensor(c_new[:], fc[:], ig[:], op=OP.add)
                nc.scalar.activation(tct[:], c_new[:], AF.Tanh)
                for w in range(2):
                    wmt = pt.tile([128, B], f32, tag="tp")
                    nc.tensor.transpose(out=wmt[:], in_=tct[:, 128 * w : 128 * (w + 1)],
                                        identity=ident[0:B, 0:B])
                nc.vector.tensor_tensor(h1[:], acts[:, sO], tct[:], op=OP.mult)
                c_prev = c_new

                # ---- h1 -> h1.T tiles (PE transpose); both copies on DVE so
                # the logits (hTr) unblock no later than the gates (hT) ----
                hT = sp.tile([128, KH, B], f32, tag="hT")
                hTr = sp.tile([128, KH, B], bf16, tag="hTr")
                for k in range(KH):
                    tp = pt.tile([128, B], f32, tag="tp")
                    nc.tensor.transpose(out=tp[:], in_=h1[:, k * 128 : (k + 1) * 128],
                                        identity=ident[0:B, 0:B])
                    nc.vector.tensor_copy(hTr[:, k, :], tp[:])
                    nc.vector.tensor_copy(hT[:, k, :], tp[:])

                # ---- logits shard in bf16 + per-tile top-8 scan ----
                # scalar drains PSUM -> SBUF, GpSimd adds b_out in place,
                # MAX8/MAX_INDEX8 on the biased f32 SBUF tile (exact f32
                # values, so no duplicate-tie hazard); a second scalar copy
                # mirrors each tile to bf16 so the output write is half-size
                logits = kp.tile([B, VS], f32, tag="logits")
                last = t == steps - 1
                for n in range(NT):
                    ns = slice(n * TN, (n + 1) * TN)
                    lg = pl.tile([B, TN], f32, tag="lg")
                    for k in range(KH):
                        nc.tensor.matmul(out=lg[:], lhsT=hTr[:, k, :], rhs=wout[:, k, ns],
                                         start=(k == 0), stop=(k == KH - 1))
                    nc.scalar.activation(logits[:, ns], lg[:], AF.Copy)
                    nc.gpsimd.tensor_tensor(logits[:, ns], logits[:, ns], biasb[:, ns], op=OP.add)
                    if not last:
                        nc.vector.max(out=m8all[:, n, :], in_=logits[:, ns])
                        nc.vector.max_index(out=i8all[:, n, :], in_max=m8all[:, n, :],
                                            in_values=logits[:, ns])
                if last:
                    nc.scalar.dma_start(out=out_p[t], in_=logits[:])
                    break

                # ---- gates h-part for t+1 (exact fp32) — PE works under the tail ----
                gates = pg.tile([B, G4], f32, tag="gates")
                for n in range(4):
                    ns = slice(n * 512, (n + 1) * 512)
                    for k in range(KH):
                        nc.tensor.matmul(out=gates[:, ns], lhsT=hT[:, k, :], rhs=whh[:, k, ns],
                                         start=(k == 0), stop=(k == KH - 1))

                # ---- local top-2 (distinct ids) from the 24 pooled candidates;
                # each candidate's W_out row gather is issued as soon as its id
                # is known so the transfer overlaps the next selection round ----
                # top-2 by the sorted pooled values: slot k keeps entries in
                # the value band [m8l[k], m8l[k-1]) via (v is_lt m8l[k])*BIG +
                # (v is_ge m8l[k-1])*BIG + id, then min-reduce. Both chains
                # depend only on m8l, so the candidate-0 row gather issues
                # while the candidate-1 selection still runs. (Exact f32 value
                # ties across distinct ids are ~impossible.)
                tv = m8all[:, :, 0:TPT]
                nc.vector.tensor_copy(tif[:], i8all[:, :, 0:TPT])
                nc.vector.tensor_tensor(tif[:], tif[:], tbase[:].rearrange("b (n c) -> b n c", c=TPT), op=OP.add)
                nc.vector.max(out=m8l[:], in_=tv)
                for k in range(NCAND):
                    nc.vector.tensor_scalar(dcand[:], tv, m8l[:, k : k + 1], BIG, op0=OP.is_lt, op1=OP.mult)
                    if k > 0:
                        nc.vector.tensor_scalar(dhi[:], tv, m8l[:, k - 1 : k], BIG, op0=OP.is_ge, op1=OP.mult)
                        nc.vector.tensor_tensor(dcand[:], dcand[:], dhi[:], op=OP.add)
                    nc.vector.tensor_tensor(dcand[:], tif[:], dcand[:], op=OP.add)
                    nc.vector.tensor_reduce(si[:, k : k + 1], dcand[:], axis=AX.XY, op=OP.min)
                    nc.vector.tensor_copy(cu[:, k : k + 1], si[:, k : k + 1])
                    nc.gpsimd.indirect_dma_start(
                        out=wrow3[:, k], out_offset=None, in_=wfullb_p[:],
                        in_offset=bass.IndirectOffsetOnAxis(ap=cu[:, k : k + 1], axis=0),
                    )
                nc.vector.tensor_copy(pk[:, NCAND : 2 * NCAND], si[:])

                # ---- exact fp32 recheck of the candidates; per-candidate
                # reduce so candidate 0's dot finishes while candidate 1's
                # row gather is still in flight ----
                for j in range(NCAND):
                    nc.vector.tensor_tensor(prod3[:, j], wrow3[:, j], h1a[:], op=OP.mult)
                    nc.vector.tensor_reduce(pk[:, j : j + 1], prod3[:, j], axis=AX.X, op=OP.add)

                # ---- AllGather of (2 exact vals | 2 ids) per core ----
                nc.vector.transpose(pkT[:], pk[:])
                agin = dp.tile([2 * NCAND, B], f32, tag="agin")
                nc.sync.dma_start(out=agin[:, 0:32], in_=pkT[0 : 2 * NCAND, 0:32])
                nc.scalar.dma_start(out=agin[:, 32:64], in_=pkT[32 : 32 + 2 * NCAND, 0:32])
                # the logits write is issued on Sync here so most of its
                # transfers ride inside the collective window
                nc.sync.dma_start(out=out_p[t], in_=logits[:])
                agout = dp.tile([NCORES * 2 * NCAND, B], f32, tag="agout", addr_space="Shared")
                nc.gpsimd.collective_compute(
                    "AllGather", OP.bypass, replica_groups=rg,
                    ins=[agin[:].opt()], outs=[agout[:].opt()],
                )
                # block-swapped readback so one DVE stream-transpose finishes it
                nc.sync.dma_start(out=gpre[0:32, 0:32], in_=agout[0:32, 0:32])
                nc.scalar.dma_start(out=gpre[32:64, 0:32], in_=agout[0:32, 32:64])
                nc.vector.transpose(gpT[:], gpre[:])
                gall = gpT[:, 0:32].rearrange("b (r s) -> b r s", s=2 * NCAND)
                gv = gall[:, :, 0:NCAND]
                gi = gall[:, :, NCAND : 2 * NCAND]

                # ---- global argmax over 16 exact candidates (min-id tiebreak) ----
                nc.vector.max(out=gm8[:], in_=gv)
                nc.vector.tensor_scalar(msk[:], gv, gm8[:, 0:1], BIG, op0=OP.is_lt, op1=OP.mult)
                nc.vector.tensor_tensor(msk[:], gi, msk[:], op=OP.add)
                nc.vector.tensor_reduce(gidxf[:], msk[:], axis=AX.XY, op=OP.min)
                nc.vector.tensor_copy(gidx[:], gidxf[:])

                # ---- gather the token's precomputed gate row in two halves:
                # [f,g] lands first so the pointwise adds start early ----
                nc.gpsimd.indirect_dma_start(
                    out=gx[:, 0:HG], out_offset=None, in_=whe0_p[:],
                    in_offset=bass.IndirectOffsetOnAxis(ap=gidx[:, :1], axis=0),
                )
                nc.gpsimd.indirect_dma_start(
                    out=gx[:, HG:G4], out_offset=None, in_=whe1_p[:],
                    in_offset=bass.IndirectOffsetOnAxis(ap=gidx[:, :1], axis=0),
                )

    nc.finalize()  # Bacc: runs compile() legalization passes
    return nc


# gate-unit permutation [f, g, i, o] (torch order in the weights is i, f, g, o)
_PERM = np.concatenate([np.arange(512, 1024), np.arange(1024, 1536),
                        np.arange(0, 512), np.arange(1536, 2048)])


def make_in_maps(inputs):
    inp = {k: np.asarray(v) for k, v in inputs.items()}
    h0 = inp["h0"].astype(np.float32)
    c0 = inp["c0"].astype(np.float32)
    W_ih = inp["W_ih"].astype(np.float32)
    W_hh = inp["W_hh"].astype(np.float32)
    b = (inp["b_ih"].astype(np.float32) + inp["b_hh"].astype(np.float32))
    W_out = inp["W_out"].astype(np.float32)
    b_out = inp["b_out"].astype(np.float32)
    emb = inp["embed_table"].astype(np.float32)
    # x @ W_ih.T + b for every vocab row, fp32, gate units permuted to [f,g,i,o]
    whe = np.ascontiguousarray((emb @ W_ih.T + b)[:, _PERM].astype(np.float32))
    whe0 = np.ascontiguousarray(whe[:, 0 : G4 // 2])
    whe1 = np.ascontiguousarray(whe[:, G4 // 2 : G4])
    wfullb = np.ascontiguousarray(
        np.concatenate([W_out, b_out.reshape(V, 1)], axis=1).astype(np.float32))
    whhT = np.ascontiguousarray(W_hh[_PERM].T)
    bias = np.ascontiguousarray(b[_PERM].reshape(1, G4))
    in_maps = []
    for c in range(NCORES):
        base = c * VS
        tbase = np.zeros((B, NT * TPT), np.float32)
        for n in range(NT):
            tbase[:, TPT * n : TPT * (n + 1)] = float(base + n * TN)
        in_maps.append({
            "h0T": np.ascontiguousarray(h0.T),
            "c0": np.ascontiguousarray(c0),
            "whhT": whhT,
            "bias": bias,
            "woutT": np.ascontiguousarray(W_out[base : base + VS].T),
            "bout": np.ascontiguousarray(b_out[base : base + VS].reshape(1, VS)),
            "whe0": whe0,
            "whe1": whe1,
            "wfullb": wfullb,
            "tbase": tbase,
        })
    return in_maps


def run(inputs, steps=S, trace=False):
    from concourse.bass_utils import run_bass_kernel_spmd

    nc = build_program(steps)
    res = run_bass_kernel_spmd(nc, make_in_maps(inputs), list(range(NCORES)),
                               trace=trace)
    outs = [np.asarray(res.results[c]["out"]).astype(np.float32)
            for c in range(NCORES)]                            # each [steps, B, VS]
    full = np.concatenate(outs, axis=2)                        # [steps, B, V]
    return np.ascontiguousarray(np.transpose(full, (1, 0, 2))), res


def kernel(**inputs):
    out, _ = run(inputs, steps=S, trace=False)
    return out.astype(np.float32)


# revision 38
# speedup vs baseline: 1.0883x; 1.0216x over previous
"""LSTM greedy decoder on 8 trn2 NeuronCores.

Vocab-parallel: each core keeps a resident fp32r SBUF copy of its
4000-row W_out shard, replicates the LSTM cell (exact fp32), and agrees
on the greedy token by exchanging per-core top-2 exact candidates via a
tiny AllGather.

Scheduling notes (r1):
- per-tile MAX8/MAX_INDEX8 run pipelined under the logits matmuls and
  write straight into persistent [B, NT, 8] slots (no per-tile copies).
- the b_out bias-add runs on GpSimd reading the PSUM tile directly and
  writing the SBUF logits tile (the scalar PSUM->SBUF copy is gone).
- the top-2 candidate logits are re-evaluated exactly in fp32 BEFORE
  the AllGather; each candidate's W_out row gather is issued as soon as
  its id is known (overlapping the next selection round), with a
  per-candidate mult+reduce so candidate 0's dot finishes while
  candidate 1's row gather is still in flight.
- AllGather staging uses DVE 32x32 stream-transposes + contiguous DMAs.
- the whe token-row gather is split in two column halves ([f,g] then
  [i,o]) so the LSTM pointwise can start on the first half early.
- LSTM pointwise is choreographed across engines: per-gate adds on
  Vector (+o on GpSimd), activations on Scalar in dependency order
  (sigmoid f, tanh g, sigmoid i, sigmoid o), fc on GpSimd.
- gate layout is host-permuted to [f, g, i, o].
- gates matmuls for step t+1 are emitted after the logits so the PE
  works under the collective tail.
- dense PE warm bursts (dummy transposes reading constants, so no data
  deps) keep the PE p-state high across the pointwise window.
- all single-buffer tail scratch tiles are allocated once, outside the
  step loop.
"""

import numpy as np

B, H, D, V, S = 64, 512, 256, 32000, 64
NCORES = 8
VS = V // NCORES            # 4000 vocab rows per core
G4 = 4 * H                  # 2048 gate units
NT = 8                      # logits N-tiles per step
TN = VS // NT               # 500 columns per logits tile
KH = H // 128               # 4 contraction tiles over H
BIG = 1.0e9
NCAND = 2                   # exact-rechecked candidates per core
TPT = 3                     # per-tile candidates pooled


def build_program(steps=S):
    import concourse.bass as bass
    import concourse.bacc as bacc
    import concourse.mybir as mybir
    import concourse.tile as tile
    from concourse.masks import make_identity

    f32 = mybir.dt.float32
    bf16 = mybir.dt.bfloat16
    u32 = mybir.dt.uint32
    AF = mybir.ActivationFunctionType
    OP = mybir.AluOpType
    AX = mybir.AxisListType

    nc = bacc.Bacc(num_devices=NCORES)
    h0T_p = nc.declare_dram_parameter("h0T", [H, B], f32, isOutput=False)
    c0_p = nc.declare_dram_parameter("c0", [B, H], f32, isOutput=False)
    whhT_p = nc.declare_dram_parameter("whhT", [H, G4], f32, isOutput=False)
    bias_p = nc.declare_dram_parameter("bias", [1, G4], f32, isOutput=False)
    woutT_p = nc.declare_dram_parameter("woutT", [H, VS], f32, isOutput=False)
    bout_p = nc.declare_dram_parameter("bout", [1, VS], f32, isOutput=False)
    whe0_p = nc.declare_dram_parameter("whe0", [V, G4 // 2], f32, isOutput=False)
    whe1_p = nc.declare_dram_parameter("whe1", [V, G4 // 2], f32, isOutput=False)
    wfullb_p = nc.declare_dram_parameter("wfullb", [V, H + 1], f32, isOutput=False)
    tbase_p = nc.declare_dram_parameter("tbase", [B, NT * TPT], f32, isOutput=False)
    out_p = nc.declare_dram_parameter("out", [steps, B, VS], f32, isOutput=True)

    rg = [list(range(NCORES))]
    HG = G4 // 2  # 1024: [f,g] | [i,o] halves
    sF = slice(0, 512)
    sG = slice(512, 1024)
    sI = slice(1024, 1536)
    sO = slice(1536, 2048)

    with tile.TileContext(nc) as tc:
        with (
            tc.tile_pool(name="wpool", bufs=1) as wp,
            tc.tile_pool(name="state", bufs=2) as sp,
            tc.tile_pool(name="work", bufs=2) as kp,
            tc.tile_pool(name="ps_g", bufs=1, space="PSUM") as pg,
            tc.tile_pool(name="ps_l", bufs=2, space="PSUM") as pl,
            tc.tile_pool(name="ps_t", bufs=2, space="PSUM") as pt,
            tc.tile_pool(name="dram", bufs=2, space="DRAM") as dp,
        ):
            # ---- constants (engine-local, no DMA) ----
            ident = wp.tile([128, 128], f32)
            make_identity(nc, ident[:])
            ones1 = wp.tile([1, B], f32)
            nc.vector.memset(ones1[:], 1.0)

            # ---- resident weights (barriers cap per-inst sync-wait fan-in) ----
            tc.strict_bb_all_engine_barrier()
            whh = wp.tile([128, KH, G4], f32)
            nc.sync.dma_start(out=whh[:], in_=whhT_p[:].rearrange("(a p) n -> p a n", p=128))
            bias = wp.tile([1, G4], f32)
            nc.sync.dma_start(out=bias[:], in_=bias_p[:])
            tbase = wp.tile([B, NT * TPT], f32)
            nc.sync.dma_start(out=tbase[:], in_=tbase_p[:])
            tc.strict_bb_all_engine_barrier()

            # bf16 logits weights: stage fp32 chunks through the logits-tagged
            # slot, then round-copy. bf16 matmul runs 1 cyc/row (vs fp32r's
            # effective 2) and at ~half the PE power, which also relieves the
            # HAM util-limit throttle windows. The exact fp32 recheck of the
            # top-2 candidates keeps the token feedback path exact.
            wout = wp.tile([128, KH, VS], bf16)
            wq = woutT_p[:].rearrange("(a p) n -> p a n", p=128)
            for k in range(KH):
                stage = kp.tile([128, VS], f32, tag="logits", name=f"wstage{k}")
                nc.sync.dma_start(out=stage[:], in_=wq[:, k, :])
                nc.vector.tensor_copy(wout[:, k, :], stage[:])
                tc.strict_bb_all_engine_barrier()
            # broadcast b_out across the 64 batch partitions once
            biasb = wp.tile([B, VS], f32)
            bstage = kp.tile([1, VS], f32, tag="logits", name="bstage")
            nc.sync.dma_start(out=bstage[:], in_=bout_p[:])
            for n in range(NT):
                ns = slice(n * TN, (n + 1) * TN)
                lg = pl.tile([B, TN], f32, tag="lg")
                nc.tensor.matmul(out=lg[:], lhsT=ones1[:], rhs=bstage[:, ns],
                                 start=True, stop=True)
                nc.scalar.activation(biasb[:, ns], lg[:], AF.Copy)
            tc.strict_bb_all_engine_barrier()

            # ---- loop-invariant scratch (alloc once: per-iter bufs=1
            # re-allocs hit the min-join fallback and stall on Tensor) ----
            pk = wp.tile([B, 32], f32)          # [ev0 ev1 | id0 id1 | pad]
            nc.vector.memset(pk[:], 0.0)
            pkT = wp.tile([B, 32], f32)
            gpre = wp.tile([B, 32], f32)
            nc.vector.memset(gpre[:], 0.0)
            gpT = wp.tile([B, 32], f32)
            m8all = wp.tile([B, NT, 8], f32)    # per-tile sorted top-8 values
            i8all = wp.tile([B, NT, 8], u32)    # per-tile top-8 indices
            tif = wp.tile([B, NT, TPT], f32)    # pooled candidate global ids
            m8l = wp.tile([B, 8], f32)
            si = wp.tile([B, NCAND], f32)
            dcand = wp.tile([B, NT, TPT], f32)
            dhi = wp.tile([B, NT, TPT], f32)
            cu = wp.tile([B, NCAND], u32)
            wrow3 = wp.tile([B, NCAND, H + 1], f32)
            prod3 = wp.tile([B, NCAND, H + 1], f32)
            gm8 = wp.tile([B, 8], f32)
            msk = wp.tile([B, NCORES, NCAND], f32)
            gidxf = wp.tile([B, 1], f32)
            gidx = wp.tile([B, 1], u32)
            gx = wp.tile([B, G4], f32)
            nc.vector.memset(gx[:], 0.0)
            gsb = wp.tile([B, G4], f32)
            acts = wp.tile([B, G4], f32)
            fc = wp.tile([B, H], f32)
            ig = wp.tile([B, H], f32)
            tct = wp.tile([B, H], f32)
            h1a = wp.tile([B, H + 1], f32)
            nc.vector.memset(h1a[:, H : H + 1], 1.0)
            h1 = h1a[:, 0:H]

            hT = sp.tile([128, KH, B], f32, tag="hT")
            nc.sync.dma_start(out=hT[:], in_=h0T_p[:].rearrange("(a p) b -> p a b", p=128))
            tc.strict_bb_all_engine_barrier()
            hTr = sp.tile([128, KH, B], bf16, tag="hTr")
            nc.vector.tensor_copy(hTr[:], hT[:])
            c_prev = sp.tile([B, H], f32, tag="c")
            nc.sync.dma_start(out=c_prev[:], in_=c0_p[:])
            tc.strict_bb_all_engine_barrier()

            # ---- gates for t=0: bias + W_hh @ h0 (exact fp32) ----
            gates = pg.tile([B, G4], f32, tag="gates")
            for n in range(4):
                ns = slice(n * 512, (n + 1) * 512)
                nc.tensor.matmul(out=gates[:, ns], lhsT=ones1[:], rhs=bias[:, ns],
                                 start=True, stop=False)
                for k in range(KH):
                    nc.tensor.matmul(out=gates[:, ns], lhsT=hT[:, k, :], rhs=whh[:, k, ns],
                                     start=False, stop=(k == KH - 1))

            first = True
            for t in range(steps):
                # ---- LSTM pointwise; gate layout is [f, g, i, o] ----
                # per-gate source: t=0 reads gates PSUM directly (no x-part);
                # later steps add the gathered x-contribution first.
                if first:
                    src = gates
                else:
                    nc.vector.tensor_tensor(gsb[:, sF], gates[:, sF], gx[:, sF], op=OP.add)
                    nc.vector.tensor_tensor(gsb[:, sG], gates[:, sG], gx[:, sG], op=OP.add)
                    nc.vector.tensor_tensor(gsb[:, sI], gates[:, sI], gx[:, sI], op=OP.add)
                    nc.vector.tensor_tensor(gsb[:, sO], gates[:, sO], gx[:, sO], op=OP.add)
                    src = gsb
                nc.scalar.activation(acts[:, sF], src[:, sF], AF.Sigmoid)
                nc.scalar.activation(acts[:, sG], src[:, sG], AF.Tanh)
                nc.scalar.activation(acts[:, sI], src[:, sI], AF.Sigmoid)
                nc.scalar.activation(acts[:, sO], src[:, sO], AF.Sigmoid)
                first = False
                # dense PE warm burst with deps staggered through the
                # pointwise window (gx -> acts -> tct) so the PE stays near
                # its high p-state right up to the hT transposes
                for w in range(4):
                    wmt = pt.tile([128, B], f32, tag="tp")
                    nc.tensor.transpose(out=wmt[:], in_=gx[:, 128 * w : 128 * (w + 1)],
                                        identity=ident[0:B, 0:B])
                for src_ap in (acts[:, 0:128], acts[:, 512:640], acts[:, 1024:1152],
                               acts[:, 1536:1664]):
                    wmt = pt.tile([128, B], f32, tag="tp")
                    nc.tensor.transpose(out=wmt[:], in_=src_ap, identity=ident[0:B, 0:B])
                nc.gpsimd.tensor_tensor(fc[:], acts[:, sF], c_prev[:], op=OP.mult)
                nc.vector.tensor_tensor(ig[:], acts[:, sI], acts[:, sG], op=OP.mult)
                c_new = sp.tile([B, H], f32, tag="c")
                nc.vector.tensor_tensor(c_new[:], fc[:], ig[:], op=OP.add)
                # ---- tct/h1 in halves so the first hTr casts land early;
                # only the bf16 hTr copy sits on the logits critical path
                # (the f32 hT for gates is re-transposed later in the tail) ----
                hTr = sp.tile([128, KH, B], bf16, tag="hTr")
                c_prev = c_new
                for hh in range(2):
                    sl = slice(hh * 256, (hh + 1) * 256)
                    nc.scalar.activation(tct[:, sl], c_new[:, sl], AF.Tanh)
                    if hh == 0:
                        for w in range(2):
                            wmt = pt.tile([128, B], f32, tag="tp")
                            nc.tensor.transpose(out=wmt[:], in_=tct[:, 128 * w : 128 * (w + 1)],
                                                identity=ident[0:B, 0:B])
                    nc.vector.tensor_tensor(h1[:, sl], acts[:, 1536 + hh * 256 : 1536 + (hh + 1) * 256],
                                            tct[:, sl], op=OP.mult)
                    for k in range(2 * hh, 2 * hh + 2):
                        tp = pt.tile([128, B], f32, tag="tp")
                        nc.tensor.transpose(out=tp[:], in_=h1[:, k * 128 : (k + 1) * 128],
                                            identity=ident[0:B, 0:B])
                        nc.vector.tensor_copy(hTr[:, k, :], tp[:])

                # ---- logits shard in bf16 + per-tile top-8 scan ----
                # scalar drains PSUM -> SBUF, GpSimd adds b_out in place,
                # MAX8/MAX_INDEX8 on the biased f32 SBUF tile (exact f32
                # values, so no duplicate-tie hazard); a second scalar copy
                # mirrors each tile to bf16 so the output write is half-size
                logits = kp.tile([B, VS], f32, tag="logits")
                last = t == steps - 1
                for n in range(NT):
                    ns = slice(n * TN, (n + 1) * TN)
                    lg = pl.tile([B, TN], f32, tag="lg")
                    for k in range(KH):
                        nc.tensor.matmul(out=lg[:], lhsT=hTr[:, k, :], rhs=wout[:, k, ns],
                                         start=(k == 0), stop=(k == KH - 1))
                    nc.scalar.activation(logits[:, ns], lg[:], AF.Copy)
                    if n % 2 == 1:
                        # paired bias-add halves the flat-cost GpSimd chain
                        np2 = slice((n - 1) * TN, (n + 1) * TN)
                        nc.gpsimd.tensor_tensor(logits[:, np2], logits[:, np2], biasb[:, np2], op=OP.add)
                        if not last:
                            for m in (n - 1, n):
                                ms = slice(m * TN, (m + 1) * TN)
                                nc.vector.max(out=m8all[:, m, :], in_=logits[:, ms])
                                nc.vector.max_index(out=i8all[:, m, :], in_max=m8all[:, m, :],
                                                    in_values=logits[:, ms])
                if last:
                    nc.scalar.dma_start(out=out_p[t], in_=logits[:])
                    break

                # ---- h1.T f32 tiles for the gates matmul, off the logits
                # critical path: PE re-transposes in the tail, scalar copies ----
                hT = sp.tile([128, KH, B], f32, tag="hT")
                for k in range(KH):
                    tp = pt.tile([128, B], f32, tag="tp")
                    nc.tensor.transpose(out=tp[:], in_=h1[:, k * 128 : (k + 1) * 128],
                                        identity=ident[0:B, 0:B])
                    nc.scalar.activation(hT[:, k, :], tp[:], AF.Copy)

                # ---- gates h-part for t+1 (exact fp32) — PE works under the tail ----
                gates = pg.tile([B, G4], f32, tag="gates")
                for n in range(4):
                    ns = slice(n * 512, (n + 1) * 512)
                    for k in range(KH):
                        nc.tensor.matmul(out=gates[:, ns], lhsT=hT[:, k, :], rhs=whh[:, k, ns],
                                         start=(k == 0), stop=(k == KH - 1))

                # ---- local top-2 (distinct ids) from the 24 pooled candidates;
                # each candidate's W_out row gather is issued as soon as its id
                # is known so the transfer overlaps the next selection round ----
                # top-2 by the sorted pooled values: slot k keeps entries in
                # the value band [m8l[k], m8l[k-1]) via (v is_lt m8l[k])*BIG +
                # (v is_ge m8l[k-1])*BIG + id, then min-reduce. Both chains
                # depend only on m8l, so the candidate-0 row gather issues
                # while the candidate-1 selection still runs. (Exact f32 value
                # ties across distinct ids are ~impossible.)
                tv = m8all[:, :, 0:TPT]
                nc.vector.tensor_copy(tif[:], i8all[:, :, 0:TPT])
                nc.vector.tensor_tensor(tif[:], tif[:], tbase[:].rearrange("b (n c) -> b n c", c=TPT), op=OP.add)
                nc.vector.max(out=m8l[:], in_=tv)
                for k in range(NCAND):
                    nc.vector.tensor_scalar(dcand[:], tv, m8l[:, k : k + 1], BIG, op0=OP.is_lt, op1=OP.mult)
                    if k > 0:
                        nc.vector.tensor_scalar(dhi[:], tv, m8l[:, k - 1 : k], BIG, op0=OP.is_ge, op1=OP.mult)
                        nc.vector.tensor_tensor(dcand[:], dcand[:], dhi[:], op=OP.add)
                    nc.vector.tensor_tensor(dcand[:], tif[:], dcand[:], op=OP.add)
                    nc.vector.tensor_reduce(si[:, k : k + 1], dcand[:], axis=AX.XY, op=OP.min)
                    nc.vector.tensor_copy(cu[:, k : k + 1], si[:, k : k + 1])
                    nc.gpsimd.indirect_dma_start(
                        out=wrow3[:, k], out_offset=None, in_=wfullb_p[:],
                        in_offset=bass.IndirectOffsetOnAxis(ap=cu[:, k : k + 1], axis=0),
                    )
                nc.vector.tensor_copy(pk[:, NCAND : 2 * NCAND], si[:])

                # ---- exact fp32 recheck of the candidates; per-candidate
                # reduce so candidate 0's dot finishes while candidate 1's
                # row gather is still in flight ----
                for j in range(NCAND):
                    nc.vector.tensor_tensor(prod3[:, j], wrow3[:, j], h1a[:], op=OP.mult)
                    nc.vector.tensor_reduce(pk[:, j : j + 1], prod3[:, j], axis=AX.X, op=OP.add)

                # ---- AllGather of (2 exact vals | 2 ids) per core ----
                nc.vector.transpose(pkT[:], pk[:])
                agin = dp.tile([2 * NCAND, B], f32, tag="agin")
                nc.sync.dma_start(out=agin[:, 0:32], in_=pkT[0 : 2 * NCAND, 0:32])
                nc.scalar.dma_start(out=agin[:, 32:64], in_=pkT[32 : 32 + 2 * NCAND, 0:32])
                # the logits write is issued on Sync here so most of its
                # transfers ride inside the collective window
                nc.sync.dma_start(out=out_p[t], in_=logits[:])
                agout = dp.tile([NCORES * 2 * NCAND, B], f32, tag="agout", addr_space="Shared")
                nc.gpsimd.collective_compute(
                    "AllGather", OP.bypass, replica_groups=rg,
                    ins=[agin[:].opt()], outs=[agout[:].opt()],
                )
                # block-swapped readback so one DVE stream-transpose finishes it
                nc.sync.dma_start(out=gpre[0:32, 0:32], in_=agout[0:32, 0:32])
                nc.scalar.dma_start(out=gpre[32:64, 0:32], in_=agout[0:32, 32:64])
                nc.vector.transpose(gpT[:], gpre[:])
                gall = gpT[:, 0:32].rearrange("b (r s) -> b r s", s=2 * NCAND)
                gv = gall[:, :, 0:NCAND]
                gi = gall[:, :, NCAND : 2 * NCAND]

                # ---- global argmax over 16 exact candidates (min-id tiebreak) ----
                nc.vector.max(out=gm8[:], in_=gv)
                nc.vector.tensor_scalar(msk[:], gv, gm8[:, 0:1], BIG, op0=OP.is_lt, op1=OP.mult)
                nc.vector.tensor_tensor(msk[:], gi, msk[:], op=OP.add)
                nc.vector.tensor_reduce(gidxf[:], msk[:], axis=AX.XY, op=OP.min)
                nc.vector.tensor_copy(gidx[:], gidxf[:])

                # ---- gather the token's precomputed gate row in two halves:
                # [f,g] lands first so the pointwise adds start early ----
                nc.gpsimd.indirect_dma_start(
                    out=gx[:, 0:HG], out_offset=None, in_=whe0_p[:],
                    in_offset=bass.IndirectOffsetOnAxis(ap=gidx[:, :1], axis=0),
                )
                nc.gpsimd.indirect_dma_start(
                    out=gx[:, HG:G4], out_offset=None, in_=whe1_p[:],
                    in_offset=bass.IndirectOffsetOnAxis(ap=gidx[:, :1], axis=0),
                )

    nc.finalize()  # Bacc: runs compile() legalization passes
    return nc


# gate-unit permutation [f, g, i, o] (torch order in the weights is i, f, g, o)
_PERM = np.concatenate([np.arange(512, 1024), np.arange(1024, 1536),
                        np.arange(0, 512), np.arange(1536, 2048)])


def make_in_maps(inputs):
    inp = {k: np.asarray(v) for k, v in inputs.items()}
    h0 = inp["h0"].astype(np.float32)
    c0 = inp["c0"].astype(np.float32)
    W_ih = inp["W_ih"].astype(np.float32)
    W_hh = inp["W_hh"].astype(np.float32)
    b = (inp["b_ih"].astype(np.float32) + inp["b_hh"].astype(np.float32))
    W_out = inp["W_out"].astype(np.float32)
    b_out = inp["b_out"].astype(np.float32)
    emb = inp["embed_table"].astype(np.float32)
    # x @ W_ih.T + b for every vocab row, fp32, gate units permuted to [f,g,i,o]
    whe = np.ascontiguousarray((emb @ W_ih.T + b)[:, _PERM].astype(np.float32))
    whe0 = np.ascontiguousarray(whe[:, 0 : G4 // 2])
    whe1 = np.ascontiguousarray(whe[:, G4 // 2 : G4])
    wfullb = np.ascontiguousarray(
        np.concatenate([W_out, b_out.reshape(V, 1)], axis=1).astype(np.float32))
    whhT = np.ascontiguousarray(W_hh[_PERM].T)
    bias = np.ascontiguousarray(b[_PERM].reshape(1, G4))
    in_maps = []
    for c in range(NCORES):
        base = c * VS
        tbase = np.zeros((B, NT * TPT), np.float32)
        for n in range(NT):
            tbase[:, TPT * n : TPT * (n + 1)] = float(base + n * TN)
        in_maps.append({
            "h0T": np.ascontiguousarray(h0.T),
            "c0": np.ascontiguousarray(c0),
            "whhT": whhT,
            "bias": bias,
            "woutT": np.ascontiguousarray(W_out[base : base + VS].T),
            "bout": np.ascontiguousarray(b_out[base : base + VS].reshape(1, VS)),
            "whe0": whe0,
            "whe1": whe1,
            "wfullb": wfullb,
            "tbase": tbase,
        })
    return in_maps


def run(inputs, steps=S, trace=False):
    from concourse.bass_utils import run_bass_kernel_spmd

    nc = build_program(steps)
    res = run_bass_kernel_spmd(nc, make_in_maps(inputs), list(range(NCORES)),
                               trace=trace)
    outs = [np.asarray(res.results[c]["out"]).astype(np.float32)
            for c in range(NCORES)]                            # each [steps, B, VS]
    full = np.concatenate(outs, axis=2)                        # [steps, B, V]
    return np.ascontiguousarray(np.transpose(full, (1, 0, 2))), res


def kernel(**inputs):
    out, _ = run(inputs, steps=S, trace=False)
    return out.astype(np.float32)


# revision 40
# speedup vs baseline: 1.0929x; 1.0043x over previous
"""LSTM greedy decoder on 8 trn2 NeuronCores.

Vocab-parallel: each core keeps a resident fp32r SBUF copy of its
4000-row W_out shard, replicates the LSTM cell (exact fp32), and agrees
on the greedy token by exchanging per-core top-2 exact candidates via a
tiny AllGather.

Scheduling notes (r1):
- per-tile MAX8/MAX_INDEX8 run pipelined under the logits matmuls and
  write straight into persistent [B, NT, 8] slots (no per-tile copies).
- the b_out bias-add runs on GpSimd reading the PSUM tile directly and
  writing the SBUF logits tile (the scalar PSUM->SBUF copy is gone).
- the top-2 candidate logits are re-evaluated exactly in fp32 BEFORE
  the AllGather; each candidate's W_out row gather is issued as soon as
  its id is known (overlapping the next selection round), with a
  per-candidate mult+reduce so candidate 0's dot finishes while
  candidate 1's row gather is still in flight.
- AllGather staging uses DVE 32x32 stream-transposes + contiguous DMAs.
- the whe token-row gather is split in two column halves ([f,g] then
  [i,o]) so the LSTM pointwise can start on the first half early.
- LSTM pointwise is choreographed across engines: per-gate adds on
  Vector (+o on GpSimd), activations on Scalar in dependency order
  (sigmoid f, tanh g, sigmoid i, sigmoid o), fc on GpSimd.
- gate layout is host-permuted to [f, g, i, o].
- gates matmuls for step t+1 are emitted after the logits so the PE
  works under the collective tail.
- dense PE warm bursts (dummy transposes reading constants, so no data
  deps) keep the PE p-state high across the pointwise window.
- all single-buffer tail scratch tiles are allocated once, outside the
  step loop.
"""

import numpy as np

B, H, D, V, S = 64, 512, 256, 32000, 64
NCORES = 8
VS = V // NCORES            # 4000 vocab rows per core
G4 = 4 * H                  # 2048 gate units
NT = 8                      # logits N-tiles per step
TN = VS // NT               # 500 columns per logits tile
KH = H // 128               # 4 contraction tiles over H
BIG = 1.0e9
NCAND = 2                   # exact-rechecked candidates per core
TPT = 3                     # per-tile candidates pooled


def build_program(steps=S):
    import concourse.bass as bass
    import concourse.bacc as bacc
    import concourse.mybir as mybir
    import concourse.tile as tile
    from concourse.masks import make_identity

    f32 = mybir.dt.float32
    bf16 = mybir.dt.bfloat16
    u32 = mybir.dt.uint32
    AF = mybir.ActivationFunctionType
    OP = mybir.AluOpType
    AX = mybir.AxisListType

    nc = bacc.Bacc(num_devices=NCORES)
    h0T_p = nc.declare_dram_parameter("h0T", [H, B], f32, isOutput=False)
    c0_p = nc.declare_dram_parameter("c0", [B, H], f32, isOutput=False)
    whhT_p = nc.declare_dram_parameter("whhT", [H, G4], f32, isOutput=False)
    bias_p = nc.declare_dram_parameter("bias", [1, G4], f32, isOutput=False)
    woutT_p = nc.declare_dram_parameter("woutT", [H, VS], f32, isOutput=False)
    bout_p = nc.declare_dram_parameter("bout", [1, VS], f32, isOutput=False)
    whe0_p = nc.declare_dram_parameter("whe0", [V, G4 // 2], f32, isOutput=False)
    whe1_p = nc.declare_dram_parameter("whe1", [V, G4 // 2], f32, isOutput=False)
    wfullb_p = nc.declare_dram_parameter("wfullb", [V, H + 1], f32, isOutput=False)
    tbase_p = nc.declare_dram_parameter("tbase", [B, NT * TPT], f32, isOutput=False)
    out_p = nc.declare_dram_parameter("out", [steps, B, VS], f32, isOutput=True)

    rg = [list(range(NCORES))]
    HG = G4 // 2  # 1024: [f,g] | [i,o] halves
    sF = slice(0, 512)
    sG = slice(512, 1024)
    sI = slice(1024, 1536)
    sO = slice(1536, 2048)

    with tile.TileContext(nc) as tc:
        with (
            tc.tile_pool(name="wpool", bufs=1) as wp,
            tc.tile_pool(name="state", bufs=2) as sp,
            tc.tile_pool(name="work", bufs=2) as kp,
            tc.tile_pool(name="ps_g", bufs=1, space="PSUM") as pg,
            tc.tile_pool(name="ps_l", bufs=2, space="PSUM") as pl,
            tc.tile_pool(name="ps_t", bufs=2, space="PSUM") as pt,
            tc.tile_pool(name="dram", bufs=2, space="DRAM") as dp,
        ):
            # ---- constants (engine-local, no DMA) ----
            ident = wp.tile([128, 128], f32)
            make_identity(nc, ident[:])
            ones1 = wp.tile([1, B], f32)
            nc.vector.memset(ones1[:], 1.0)

            # ---- resident weights (barriers cap per-inst sync-wait fan-in) ----
            tc.strict_bb_all_engine_barrier()
            whh = wp.tile([128, KH, G4], f32)
            nc.sync.dma_start(out=whh[:], in_=whhT_p[:].rearrange("(a p) n -> p a n", p=128))
            bias = wp.tile([1, G4], f32)
            nc.sync.dma_start(out=bias[:], in_=bias_p[:])
            tbase = wp.tile([B, NT * TPT], f32)
            nc.sync.dma_start(out=tbase[:], in_=tbase_p[:])
            tc.strict_bb_all_engine_barrier()

            # bf16 logits weights: stage fp32 chunks through the logits-tagged
            # slot, then round-copy. bf16 matmul runs 1 cyc/row (vs fp32r's
            # effective 2) and at ~half the PE power, which also relieves the
            # HAM util-limit throttle windows. The exact fp32 recheck of the
            # top-2 candidates keeps the token feedback path exact.
            wout = wp.tile([128, KH, VS], bf16)
            wq = woutT_p[:].rearrange("(a p) n -> p a n", p=128)
            for k in range(KH):
                stage = kp.tile([128, VS], f32, tag="logits", name=f"wstage{k}")
                nc.sync.dma_start(out=stage[:], in_=wq[:, k, :])
                nc.vector.tensor_copy(wout[:, k, :], stage[:])
                tc.strict_bb_all_engine_barrier()
            # b_out folds into the logits matmul as a leading 1-row bf16
            # accumulation (exchanged candidate values stay exact via the
            # f32 wfullb recheck rows); this keeps GpSimd out of the
            # logits/scan pipeline entirely
            ones_bf = wp.tile([1, B], bf16)
            nc.vector.memset(ones_bf[:], 1.0)
            bout_bf = wp.tile([1, VS], bf16)
            bstage = kp.tile([1, VS], f32, tag="logits", name="bstage")
            nc.sync.dma_start(out=bstage[:], in_=bout_p[:])
            nc.vector.tensor_copy(bout_bf[:], bstage[:])
            tc.strict_bb_all_engine_barrier()

            # ---- loop-invariant scratch (alloc once: per-iter bufs=1
            # re-allocs hit the min-join fallback and stall on Tensor) ----
            pk = wp.tile([B, 32], f32)          # [ev0 ev1 | id0 id1 | pad]
            nc.vector.memset(pk[:], 0.0)
            pkT = wp.tile([B, 32], f32)
            gpre = wp.tile([B, 32], f32)
            nc.vector.memset(gpre[:], 0.0)
            gpT = wp.tile([B, 32], f32)
            m8all = wp.tile([B, NT, 8], f32)    # per-tile sorted top-8 values
            i8all = wp.tile([B, NT, 8], u32)    # per-tile top-8 indices
            tif = wp.tile([B, NT, TPT], f32)    # pooled candidate global ids
            m8l = wp.tile([B, 8], f32)
            si = wp.tile([B, NCAND], f32)
            dcand = wp.tile([B, NT, TPT], f32)
            dhi = wp.tile([B, NT, TPT], f32)
            cu = wp.tile([B, NCAND], u32)
            wrow3 = wp.tile([B, NCAND, H + 1], f32)
            prod3 = wp.tile([B, NCAND, H + 1], f32)
            gm8 = wp.tile([B, 8], f32)
            msk = wp.tile([B, NCORES, NCAND], f32)
            gidxf = wp.tile([B, 1], f32)
            gidx = wp.tile([B, 1], u32)
            gx = wp.tile([B, G4], f32)
            nc.vector.memset(gx[:], 0.0)
            gsb = wp.tile([B, G4], f32)
            acts = wp.tile([B, G4], f32)
            fc = wp.tile([B, H], f32)
            ig = wp.tile([B, H], f32)
            tct = wp.tile([B, H], f32)
            h1a = wp.tile([B, H + 1], f32)
            nc.vector.memset(h1a[:, H : H + 1], 1.0)
            h1 = h1a[:, 0:H]

            hT = sp.tile([128, KH, B], f32, tag="hT")
            nc.sync.dma_start(out=hT[:], in_=h0T_p[:].rearrange("(a p) b -> p a b", p=128))
            tc.strict_bb_all_engine_barrier()
            hTr = sp.tile([128, KH, B], bf16, tag="hTr")
            nc.vector.tensor_copy(hTr[:], hT[:])
            c_prev = sp.tile([B, H], f32, tag="c")
            nc.sync.dma_start(out=c_prev[:], in_=c0_p[:])
            tc.strict_bb_all_engine_barrier()

            # ---- gates for t=0: bias + W_hh @ h0 (exact fp32) ----
            gates = pg.tile([B, G4], f32, tag="gates")
            for n in range(4):
                ns = slice(n * 512, (n + 1) * 512)
                nc.tensor.matmul(out=gates[:, ns], lhsT=ones1[:], rhs=bias[:, ns],
                                 start=True, stop=False)
                for k in range(KH):
                    nc.tensor.matmul(out=gates[:, ns], lhsT=hT[:, k, :], rhs=whh[:, k, ns],
                                     start=False, stop=(k == KH - 1))

            first = True
            for t in range(steps):
                # ---- LSTM pointwise; gate layout is [f, g, i, o] ----
                # per-gate source: t=0 reads gates PSUM directly (no x-part);
                # later steps add the gathered x-contribution first.
                if first:
                    src = gates
                else:
                    nc.vector.tensor_tensor(gsb[:, sF], gates[:, sF], gx[:, sF], op=OP.add)
                    nc.vector.tensor_tensor(gsb[:, sG], gates[:, sG], gx[:, sG], op=OP.add)
                    nc.vector.tensor_tensor(gsb[:, sI], gates[:, sI], gx[:, sI], op=OP.add)
                    nc.vector.tensor_tensor(gsb[:, sO], gates[:, sO], gx[:, sO], op=OP.add)
                    src = gsb
                nc.scalar.activation(acts[:, sF], src[:, sF], AF.Sigmoid)
                nc.scalar.activation(acts[:, sG], src[:, sG], AF.Tanh)
                nc.scalar.activation(acts[:, sI], src[:, sI], AF.Sigmoid)
                nc.scalar.activation(acts[:, sO], src[:, sO], AF.Sigmoid)
                first = False
                # dense PE warm burst with deps staggered through the
                # pointwise window (gx -> acts -> tct) so the PE stays near
                # its high p-state right up to the hT transposes
                for w in range(4):
                    wmt = pt.tile([128, B], f32, tag="tp")
                    nc.tensor.transpose(out=wmt[:], in_=gx[:, 128 * w : 128 * (w + 1)],
                                        identity=ident[0:B, 0:B])
                for src_ap in (acts[:, 0:128], acts[:, 512:640], acts[:, 1024:1152],
                               acts[:, 1536:1664]):
                    wmt = pt.tile([128, B], f32, tag="tp")
                    nc.tensor.transpose(out=wmt[:], in_=src_ap, identity=ident[0:B, 0:B])
                nc.gpsimd.tensor_tensor(fc[:], acts[:, sF], c_prev[:], op=OP.mult)
                nc.vector.tensor_tensor(ig[:], acts[:, sI], acts[:, sG], op=OP.mult)
                c_new = sp.tile([B, H], f32, tag="c")
                nc.vector.tensor_tensor(c_new[:], fc[:], ig[:], op=OP.add)
                # ---- tct/h1 in halves so the first hTr casts land early;
                # only the bf16 hTr copy sits on the logits critical path
                # (the f32 hT for gates is re-transposed later in the tail) ----
                hTr = sp.tile([128, KH, B], bf16, tag="hTr")
                c_prev = c_new
                for hh in range(2):
                    sl = slice(hh * 256, (hh + 1) * 256)
                    nc.scalar.activation(tct[:, sl], c_new[:, sl], AF.Tanh)
                    if hh == 0:
                        for w in range(2):
                            wmt = pt.tile([128, B], f32, tag="tp")
                            nc.tensor.transpose(out=wmt[:], in_=tct[:, 128 * w : 128 * (w + 1)],
                                                identity=ident[0:B, 0:B])
                    nc.vector.tensor_tensor(h1[:, sl], acts[:, 1536 + hh * 256 : 1536 + (hh + 1) * 256],
                                            tct[:, sl], op=OP.mult)
                    for k in range(2 * hh, 2 * hh + 2):
                        tp = pt.tile([128, B], f32, tag="tp")
                        nc.tensor.transpose(out=tp[:], in_=h1[:, k * 128 : (k + 1) * 128],
                                            identity=ident[0:B, 0:B])
                        nc.vector.tensor_copy(hTr[:, k, :], tp[:])

                # ---- logits shard in bf16 + per-tile top-8 scan ----
                # scalar drains PSUM -> SBUF, GpSimd adds b_out in place,
                # MAX8/MAX_INDEX8 on the biased f32 SBUF tile (exact f32
                # values, so no duplicate-tie hazard); a second scalar copy
                # mirrors each tile to bf16 so the output write is half-size
                logits = kp.tile([B, VS], f32, tag="logits")
                last = t == steps - 1
                for n in range(NT):
                    ns = slice(n * TN, (n + 1) * TN)
                    lg = pl.tile([B, TN], f32, tag="lg")
                    nc.tensor.matmul(out=lg[:], lhsT=ones_bf[:], rhs=bout_bf[:, ns],
                                     start=True, stop=False)
                    for k in range(KH):
                        nc.tensor.matmul(out=lg[:], lhsT=hTr[:, k, :], rhs=wout[:, k, ns],
                                         start=False, stop=(k == KH - 1))
                    nc.scalar.activation(logits[:, ns], lg[:], AF.Copy)
                    if not last:
                        nc.vector.max(out=m8all[:, n, :], in_=logits[:, ns])
                        nc.vector.max_index(out=i8all[:, n, :], in_max=m8all[:, n, :],
                                            in_values=logits[:, ns])
                if last:
                    nc.scalar.dma_start(out=out_p[t], in_=logits[:])
                    break

                # ---- h1.T f32 tiles for the gates matmul, off the logits
                # critical path: PE re-transposes in the tail, scalar copies ----
                hT = sp.tile([128, KH, B], f32, tag="hT")
                for k in range(KH):
                    tp = pt.tile([128, B], f32, tag="tp")
                    nc.tensor.transpose(out=tp[:], in_=h1[:, k * 128 : (k + 1) * 128],
                                        identity=ident[0:B, 0:B])
                    nc.scalar.activation(hT[:, k, :], tp[:], AF.Copy)

                # ---- gates h-part for t+1 (exact fp32) — PE works under the tail ----
                gates = pg.tile([B, G4], f32, tag="gates")
                for n in range(4):
                    ns = slice(n * 512, (n + 1) * 512)
                    for k in range(KH):
                        nc.tensor.matmul(out=gates[:, ns], lhsT=hT[:, k, :], rhs=whh[:, k, ns],
                                         start=(k == 0), stop=(k == KH - 1))

                # ---- local top-2 (distinct ids) from the 24 pooled candidates;
                # each candidate's W_out row gather is issued as soon as its id
                # is known so the transfer overlaps the next selection round ----
                # top-2 by the sorted pooled values: slot k keeps entries in
                # the value band [m8l[k], m8l[k-1]) via (v is_lt m8l[k])*BIG +
                # (v is_ge m8l[k-1])*BIG + id, then min-reduce. Both chains
                # depend only on m8l, so the candidate-0 row gather issues
                # while the candidate-1 selection still runs. (Exact f32 value
                # ties across distinct ids are ~impossible.)
                tv = m8all[:, :, 0:TPT]
                nc.vector.tensor_copy(tif[:], i8all[:, :, 0:TPT])
                nc.vector.tensor_tensor(tif[:], tif[:], tbase[:].rearrange("b (n c) -> b n c", c=TPT), op=OP.add)
                nc.vector.max(out=m8l[:], in_=tv)
                for k in range(NCAND):
                    nc.vector.tensor_scalar(dcand[:], tv, m8l[:, k : k + 1], BIG, op0=OP.is_lt, op1=OP.mult)
                    if k > 0:
                        nc.vector.tensor_scalar(dhi[:], tv, m8l[:, k - 1 : k], BIG, op0=OP.is_ge, op1=OP.mult)
                        nc.vector.tensor_tensor(dcand[:], dcand[:], dhi[:], op=OP.add)
                    nc.vector.tensor_tensor(dcand[:], tif[:], dcand[:], op=OP.add)
                    nc.vector.tensor_reduce(si[:, k : k + 1], dcand[:], axis=AX.XY, op=OP.min)
                    nc.vector.tensor_copy(cu[:, k : k + 1], si[:, k : k + 1])
                    nc.gpsimd.indirect_dma_start(
                        out=wrow3[:, k], out_offset=None, in_=wfullb_p[:],
                        in_offset=bass.IndirectOffsetOnAxis(ap=cu[:, k : k + 1], axis=0),
                    )
                nc.vector.tensor_copy(pk[:, NCAND : 2 * NCAND], si[:])

                # ---- exact fp32 recheck of the candidates; per-candidate
                # reduce so candidate 0's dot finishes while candidate 1's
                # row gather is still in flight ----
                for j in range(NCAND):
                    nc.vector.tensor_tensor(prod3[:, j], wrow3[:, j], h1a[:], op=OP.mult)
                    nc.vector.tensor_reduce(pk[:, j : j + 1], prod3[:, j], axis=AX.X, op=OP.add)

                # ---- AllGather of (2 exact vals | 2 ids) per core ----
                nc.vector.transpose(pkT[:], pk[:])
                agin = dp.tile([2 * NCAND, B], f32, tag="agin")
                nc.sync.dma_start(out=agin[:, 0:32], in_=pkT[0 : 2 * NCAND, 0:32])
                nc.scalar.dma_start(out=agin[:, 32:64], in_=pkT[32 : 32 + 2 * NCAND, 0:32])
                # the logits write is issued on Sync here so most of its
                # transfers ride inside the collective window
                nc.sync.dma_start(out=out_p[t], in_=logits[:])
                agout = dp.tile([NCORES * 2 * NCAND, B], f32, tag="agout", addr_space="Shared")
                nc.gpsimd.collective_compute(
                    "AllGather", OP.bypass, replica_groups=rg,
                    ins=[agin[:].opt()], outs=[agout[:].opt()],
                )
                # block-swapped readback so one DVE stream-transpose finishes it
                nc.sync.dma_start(out=gpre[0:32, 0:32], in_=agout[0:32, 0:32])
                nc.scalar.dma_start(out=gpre[32:64, 0:32], in_=agout[0:32, 32:64])
                nc.vector.transpose(gpT[:], gpre[:])
                gall = gpT[:, 0:32].rearrange("b (r s) -> b r s", s=2 * NCAND)
                gv = gall[:, :, 0:NCAND]
                gi = gall[:, :, NCAND : 2 * NCAND]

                # ---- global argmax over 16 exact candidates (min-id tiebreak) ----
                nc.vector.max(out=gm8[:], in_=gv)
                nc.vector.tensor_scalar(msk[:], gv, gm8[:, 0:1], BIG, op0=OP.is_lt, op1=OP.mult)
                nc.vector.tensor_tensor(msk[:], gi, msk[:], op=OP.add)
                nc.vector.tensor_reduce(gidxf[:], msk[:], axis=AX.XY, op=OP.min)
                nc.vector.tensor_copy(gidx[:], gidxf[:])

                # ---- gather the token's precomputed gate row in two halves:
                # [f,g] lands first so the pointwise adds start early ----
                nc.gpsimd.indirect_dma_start(
                    out=gx[:, 0:HG], out_offset=None, in_=whe0_p[:],
                    in_offset=bass.IndirectOffsetOnAxis(ap=gidx[:, :1], axis=0),
                )
                nc.gpsimd.indirect_dma_start(
                    out=gx[:, HG:G4], out_offset=None, in_=whe1_p[:],
                    in_offset=bass.IndirectOffsetOnAxis(ap=gidx[:, :1], axis=0),
                )

    nc.finalize()  # Bacc: runs compile() legalization passes
    return nc


# gate-unit permutation [f, g, i, o] (torch order in the weights is i, f, g, o)
_PERM = np.concatenate([np.arange(512, 1024), np.arange(1024, 1536),
                        np.arange(0, 512), np.arange(1536, 2048)])


def make_in_maps(inputs):
    inp = {k: np.asarray(v) for k, v in inputs.items()}
    h0 = inp["h0"].astype(np.float32)
    c0 = inp["c0"].astype(np.float32)
    W_ih = inp["W_ih"].astype(np.float32)
    W_hh = inp["W_hh"].astype(np.float32)
    b = (inp["b_ih"].astype(np.float32) + inp["b_hh"].astype(np.float32))
    W_out = inp["W_out"].astype(np.float32)
    b_out = inp["b_out"].astype(np.float32)
    emb = inp["embed_table"].astype(np.float32)
    # x @ W_ih.T + b for every vocab row, fp32, gate units permuted to [f,g,i,o]
    whe = np.ascontiguousarray((emb @ W_ih.T + b)[:, _PERM].astype(np.float32))
    whe0 = np.ascontiguousarray(whe[:, 0 : G4 // 2])
    whe1 = np.ascontiguousarray(whe[:, G4 // 2 : G4])
    wfullb = np.ascontiguousarray(
        np.concatenate([W_out, b_out.reshape(V, 1)], axis=1).astype(np.float32))
    whhT = np.ascontiguousarray(W_hh[_PERM].T)
    bias = np.ascontiguousarray(b[_PERM].reshape(1, G4))
    in_maps = []
    for c in range(NCORES):
        base = c * VS
        tbase = np.zeros((B, NT * TPT), np.float32)
        for n in range(NT):
            tbase[:, TPT * n : TPT * (n + 1)] = float(base + n * TN)
        in_maps.append({
            "h0T": np.ascontiguousarray(h0.T),
            "c0": np.ascontiguousarray(c0),
            "whhT": whhT,
            "bias": bias,
            "woutT": np.ascontiguousarray(W_out[base : base + VS].T),
            "bout": np.ascontiguousarray(b_out[base : base + VS].reshape(1, VS)),
            "whe0": whe0,
            "whe1": whe1,
            "wfullb": wfullb,
            "tbase": tbase,
        })
    return in_maps


def run(inputs, steps=S, trace=False):
    from concourse.bass_utils import run_bass_kernel_spmd

    nc = build_program(steps)
    res = run_bass_kernel_spmd(nc, make_in_maps(inputs), list(range(NCORES)),
                               trace=trace)
    outs = [np.asarray(res.results[c]["out"]).astype(np.float32)
            for c in range(NCORES)]                            # each [steps, B, VS]
    full = np.concatenate(outs, axis=2)                        # [steps, B, V]
    return np.ascontiguousarray(np.transpose(full, (1, 0, 2))), res


def kernel(**inputs):
    out, _ = run(inputs, steps=S, trace=False)
    return out.astype(np.float32)


# revision 44
# speedup vs baseline: 1.1367x; 1.0401x over previous
"""LSTM greedy decoder on 8 trn2 NeuronCores.

Vocab-parallel: each core keeps a resident fp32r SBUF copy of its
4000-row W_out shard, replicates the LSTM cell (exact fp32), and agrees
on the greedy token by exchanging per-core top-2 exact candidates via a
tiny AllGather.

Scheduling notes (r1):
- per-tile MAX8/MAX_INDEX8 run pipelined under the logits matmuls and
  write straight into persistent [B, NT, 8] slots (no per-tile copies).
- the b_out bias-add runs on GpSimd reading the PSUM tile directly and
  writing the SBUF logits tile (the scalar PSUM->SBUF copy is gone).
- the top-2 candidate logits are re-evaluated exactly in fp32 BEFORE
  the AllGather; each candidate's W_out row gather is issued as soon as
  its id is known (overlapping the next selection round), with a
  per-candidate mult+reduce so candidate 0's dot finishes while
  candidate 1's row gather is still in flight.
- AllGather staging uses DVE 32x32 stream-transposes + contiguous DMAs.
- the whe token-row gather is split in two column halves ([f,g] then
  [i,o]) so the LSTM pointwise can start on the first half early.
- LSTM pointwise is choreographed across engines: per-gate adds on
  Vector (+o on GpSimd), activations on Scalar in dependency order
  (sigmoid f, tanh g, sigmoid i, sigmoid o), fc on GpSimd.
- gate layout is host-permuted to [f, g, i, o].
- gates matmuls for step t+1 are emitted after the logits so the PE
  works under the collective tail.
- dense PE warm bursts (dummy transposes reading constants, so no data
  deps) keep the PE p-state high across the pointwise window.
- all single-buffer tail scratch tiles are allocated once, outside the
  step loop.
"""

import numpy as np

B, H, D, V, S = 64, 512, 256, 32000, 64
NCORES = 8
VS = V // NCORES            # 4000 vocab rows per core
G4 = 4 * H                  # 2048 gate units
NT = 8                      # logits N-tiles per step
TN = VS // NT               # 500 columns per logits tile
KH = H // 128               # 4 contraction tiles over H
BIG = 1.0e9
NCAND = 2                   # exact-rechecked candidates per core
TPT = 3                     # per-tile candidates pooled


def build_program(steps=S):
    import concourse.bass as bass
    import concourse.bacc as bacc
    import concourse.mybir as mybir
    import concourse.tile as tile
    from concourse.masks import make_identity

    f32 = mybir.dt.float32
    bf16 = mybir.dt.bfloat16
    u32 = mybir.dt.uint32
    AF = mybir.ActivationFunctionType
    OP = mybir.AluOpType
    AX = mybir.AxisListType

    nc = bacc.Bacc(num_devices=NCORES)
    h0T_p = nc.declare_dram_parameter("h0T", [H, B], f32, isOutput=False)
    c0_p = nc.declare_dram_parameter("c0", [B, H], f32, isOutput=False)
    whhT_p = nc.declare_dram_parameter("whhT", [H, G4], f32, isOutput=False)
    bias_p = nc.declare_dram_parameter("bias", [1, G4], f32, isOutput=False)
    woutT_p = nc.declare_dram_parameter("woutT", [H, VS], f32, isOutput=False)
    bout_p = nc.declare_dram_parameter("bout", [1, VS], f32, isOutput=False)
    whe0_p = nc.declare_dram_parameter("whe0", [V, G4 // 2], f32, isOutput=False)
    whe1_p = nc.declare_dram_parameter("whe1", [V, G4 // 2], f32, isOutput=False)
    wfullb_p = nc.declare_dram_parameter("wfullb", [V, H + 1], f32, isOutput=False)
    tbase_p = nc.declare_dram_parameter("tbase", [B, NT * TPT], f32, isOutput=False)
    out_p = nc.declare_dram_parameter("out", [steps, B, VS], bf16, isOutput=True)

    rg = [list(range(NCORES))]
    HG = G4 // 2  # 1024: [f,g] | [i,o] halves
    sF = slice(0, 512)
    sG = slice(512, 1024)
    sI = slice(1024, 1536)
    sO = slice(1536, 2048)

    with tile.TileContext(nc) as tc:
        with (
            tc.tile_pool(name="wpool", bufs=1) as wp,
            tc.tile_pool(name="state", bufs=2) as sp,
            tc.tile_pool(name="work", bufs=2) as kp,
            tc.tile_pool(name="ps_g", bufs=1, space="PSUM") as pg,
            tc.tile_pool(name="ps_l", bufs=2, space="PSUM") as pl,
            tc.tile_pool(name="ps_t", bufs=2, space="PSUM") as pt,
            tc.tile_pool(name="dram", bufs=2, space="DRAM") as dp,
        ):
            # ---- constants (engine-local, no DMA) ----
            ident = wp.tile([128, 128], f32)
            make_identity(nc, ident[:])
            ones1 = wp.tile([1, B], f32)
            nc.vector.memset(ones1[:], 1.0)

            # ---- resident weights (barriers cap per-inst sync-wait fan-in) ----
            tc.strict_bb_all_engine_barrier()
            whh = wp.tile([128, KH, G4], f32)
            nc.sync.dma_start(out=whh[:], in_=whhT_p[:].rearrange("(a p) n -> p a n", p=128))
            bias = wp.tile([1, G4], f32)
            nc.sync.dma_start(out=bias[:], in_=bias_p[:])
            tbase = wp.tile([B, NT * TPT], f32)
            nc.sync.dma_start(out=tbase[:], in_=tbase_p[:])
            tc.strict_bb_all_engine_barrier()

            # bf16 logits weights: stage fp32 chunks through the logits-tagged
            # slot, then round-copy. bf16 matmul runs 1 cyc/row (vs fp32r's
            # effective 2) and at ~half the PE power, which also relieves the
            # HAM util-limit throttle windows. The exact fp32 recheck of the
            # top-2 candidates keeps the token feedback path exact.
            wout = wp.tile([128, KH, VS], bf16)
            wq = woutT_p[:].rearrange("(a p) n -> p a n", p=128)
            for k in range(KH):
                stage = kp.tile([128, VS], f32, tag="logits", name=f"wstage{k}")
                nc.sync.dma_start(out=stage[:], in_=wq[:, k, :])
                nc.vector.tensor_copy(wout[:, k, :], stage[:])
                tc.strict_bb_all_engine_barrier()
            # b_out folds into the logits matmul as a leading 1-row bf16
            # accumulation (exchanged candidate values stay exact via the
            # f32 wfullb recheck rows); this keeps GpSimd out of the
            # logits/scan pipeline entirely
            ones_bf = wp.tile([1, B], bf16)
            nc.vector.memset(ones_bf[:], 1.0)
            bout_bf = wp.tile([1, VS], bf16)
            bstage = kp.tile([1, VS], f32, tag="logits", name="bstage")
            nc.sync.dma_start(out=bstage[:], in_=bout_p[:])
            nc.vector.tensor_copy(bout_bf[:], bstage[:])
            tc.strict_bb_all_engine_barrier()

            # ---- loop-invariant scratch (alloc once: per-iter bufs=1
            # re-allocs hit the min-join fallback and stall on Tensor) ----
            pk = wp.tile([B, 32], f32)          # [ev0 ev1 | id0 id1 | pad]
            nc.vector.memset(pk[:], 0.0)
            pkT = wp.tile([B, 32], f32)
            gpre = wp.tile([B, 32], f32)
            nc.vector.memset(gpre[:], 0.0)
            gpT = wp.tile([B, 32], f32)
            m8all = wp.tile([B, NT, 8], f32)    # per-tile sorted top-8 values
            i8all = wp.tile([B, NT, 8], u32)    # per-tile top-8 indices
            tif = wp.tile([B, NT, TPT], f32)    # pooled candidate global ids
            m8l = wp.tile([B, 8], f32)
            si = wp.tile([B, NCAND], f32)
            dcand = wp.tile([B, NT, TPT], f32)
            dhi = wp.tile([B, NT, TPT], f32)
            cu = wp.tile([B, NCAND], u32)
            wrow3 = wp.tile([B, NCAND, H + 1], f32)
            prod3 = wp.tile([B, NCAND, H + 1], f32)
            gm8 = wp.tile([B, 8], f32)
            msk = wp.tile([B, NCORES, NCAND], f32)
            gidxf = wp.tile([B, 1], f32)
            gidx = wp.tile([B, 1], u32)
            gx = wp.tile([B, G4], f32)
            nc.vector.memset(gx[:], 0.0)
            gsb = wp.tile([B, G4], f32)
            acts = wp.tile([B, G4], f32)
            fc = wp.tile([B, H], f32)
            ig = wp.tile([B, H], f32)
            tct = wp.tile([B, H], f32)
            h1a = wp.tile([B, H + 1], f32)
            nc.vector.memset(h1a[:, H : H + 1], 1.0)
            h1 = h1a[:, 0:H]

            hT = sp.tile([128, KH, B], f32, tag="hT")
            nc.sync.dma_start(out=hT[:], in_=h0T_p[:].rearrange("(a p) b -> p a b", p=128))
            tc.strict_bb_all_engine_barrier()
            hTr = sp.tile([128, KH, B], bf16, tag="hTr")
            nc.vector.tensor_copy(hTr[:], hT[:])
            c_prev = sp.tile([B, H], f32, tag="c")
            nc.sync.dma_start(out=c_prev[:], in_=c0_p[:])
            tc.strict_bb_all_engine_barrier()

            # ---- gates for t=0: bias + W_hh @ h0 (exact fp32) ----
            gates = pg.tile([B, G4], f32, tag="gates")
            for n in range(4):
                ns = slice(n * 512, (n + 1) * 512)
                nc.tensor.matmul(out=gates[:, ns], lhsT=ones1[:], rhs=bias[:, ns],
                                 start=True, stop=False)
                for k in range(KH):
                    nc.tensor.matmul(out=gates[:, ns], lhsT=hT[:, k, :], rhs=whh[:, k, ns],
                                     start=False, stop=(k == KH - 1))

            first = True
            for t in range(steps):
                # ---- LSTM pointwise; gate layout is [f, g, i, o] ----
                # per-gate source: t=0 reads gates PSUM directly (no x-part);
                # later steps add the gathered x-contribution first.
                if first:
                    src = gates
                else:
                    nc.vector.tensor_tensor(gsb[:, sF], gates[:, sF], gx[:, sF], op=OP.add)
                    nc.vector.tensor_tensor(gsb[:, sG], gates[:, sG], gx[:, sG], op=OP.add)
                    nc.vector.tensor_tensor(gsb[:, sI], gates[:, sI], gx[:, sI], op=OP.add)
                    nc.vector.tensor_tensor(gsb[:, sO], gates[:, sO], gx[:, sO], op=OP.add)
                    src = gsb
                nc.scalar.activation(acts[:, sF], src[:, sF], AF.Sigmoid)
                nc.scalar.activation(acts[:, sG], src[:, sG], AF.Tanh)
                nc.scalar.activation(acts[:, sI], src[:, sI], AF.Sigmoid)
                nc.scalar.activation(acts[:, sO], src[:, sO], AF.Sigmoid)
                first = False
                # dense PE warm burst with deps staggered through the
                # pointwise window (gx -> acts -> tct) so the PE stays near
                # its high p-state right up to the hT transposes
                for w in range(4):
                    wmt = pt.tile([128, B], f32, tag="tp")
                    nc.tensor.transpose(out=wmt[:], in_=gx[:, 128 * w : 128 * (w + 1)],
                                        identity=ident[0:B, 0:B])
                for src_ap in (acts[:, 0:128], acts[:, 512:640], acts[:, 1024:1152],
                               acts[:, 1536:1664]):
                    wmt = pt.tile([128, B], f32, tag="tp")
                    nc.tensor.transpose(out=wmt[:], in_=src_ap, identity=ident[0:B, 0:B])
                nc.gpsimd.tensor_tensor(fc[:], acts[:, sF], c_prev[:], op=OP.mult)
                nc.vector.tensor_tensor(ig[:], acts[:, sI], acts[:, sG], op=OP.mult)
                c_new = sp.tile([B, H], f32, tag="c")
                nc.vector.tensor_tensor(c_new[:], fc[:], ig[:], op=OP.add)
                # ---- tct/h1 in halves so the first hTr casts land early;
                # only the bf16 hTr copy sits on the logits critical path
                # (the f32 hT for gates is re-transposed later in the tail) ----
                hTr = sp.tile([128, KH, B], bf16, tag="hTr")
                c_prev = c_new
                for hh in range(2):
                    sl = slice(hh * 256, (hh + 1) * 256)
                    nc.scalar.activation(tct[:, sl], c_new[:, sl], AF.Tanh)
                    if hh == 0:
                        for w in range(2):
                            wmt = pt.tile([128, B], f32, tag="tp")
                            nc.tensor.transpose(out=wmt[:], in_=tct[:, 128 * w : 128 * (w + 1)],
                                                identity=ident[0:B, 0:B])
                    nc.vector.tensor_tensor(h1[:, sl], acts[:, 1536 + hh * 256 : 1536 + (hh + 1) * 256],
                                            tct[:, sl], op=OP.mult)
                    for k in range(2 * hh, 2 * hh + 2):
                        tp = pt.tile([128, B], f32, tag="tp")
                        nc.tensor.transpose(out=tp[:], in_=h1[:, k * 128 : (k + 1) * 128],
                                            identity=ident[0:B, 0:B])
                        nc.vector.tensor_copy(hTr[:, k, :], tp[:])

                # ---- logits shard in bf16 + per-tile top-8 scan ----
                # scalar drains PSUM -> SBUF, GpSimd adds b_out in place,
                # MAX8/MAX_INDEX8 on the biased f32 SBUF tile (exact f32
                # values, so no duplicate-tie hazard); a second scalar copy
                # mirrors each tile to bf16 so the output write is half-size
                logits = kp.tile([B, VS], f32, tag="logits")
                outb = kp.tile([B, VS], bf16, tag="outb")
                last = t == steps - 1
                for n in range(NT):
                    ns = slice(n * TN, (n + 1) * TN)
                    lg = pl.tile([B, TN], f32, tag="lg")
                    nc.tensor.matmul(out=lg[:], lhsT=ones_bf[:], rhs=bout_bf[:, ns],
                                     start=True, stop=False)
                    for k in range(KH):
                        nc.tensor.matmul(out=lg[:], lhsT=hTr[:, k, :], rhs=wout[:, k, ns],
                                         start=False, stop=(k == KH - 1))
                    nc.scalar.activation(logits[:, ns], lg[:], AF.Copy)
                    # bf16 mirror for the half-size output write; with the
                    # GpSimd bias-adds gone, Scalar (copy+mirror 1.34us/tile)
                    # stays under the Vector scan pace, and the out-DMA's dep
                    # on outb delays its transfers past the recheck gathers
                    nc.scalar.activation(outb[:, ns], logits[:, ns], AF.Copy)
                    if not last:
                        nc.vector.max(out=m8all[:, n, :], in_=logits[:, ns])
                        nc.vector.max_index(out=i8all[:, n, :], in_max=m8all[:, n, :],
                                            in_values=logits[:, ns])
                if last:
                    nc.scalar.dma_start(out=out_p[t], in_=outb[:])
                    break

                # ---- h1.T f32 tiles for the gates matmul, off the logits
                # critical path: PE re-transposes in the tail, scalar copies ----
                hT = sp.tile([128, KH, B], f32, tag="hT")
                for k in range(KH):
                    tp = pt.tile([128, B], f32, tag="tp")
                    nc.tensor.transpose(out=tp[:], in_=h1[:, k * 128 : (k + 1) * 128],
                                        identity=ident[0:B, 0:B])
                    nc.scalar.activation(hT[:, k, :], tp[:], AF.Copy)

                # ---- gates h-part for t+1 (exact fp32) — PE works under the tail ----
                gates = pg.tile([B, G4], f32, tag="gates")
                for n in range(4):
                    ns = slice(n * 512, (n + 1) * 512)
                    for k in range(KH):
                        nc.tensor.matmul(out=gates[:, ns], lhsT=hT[:, k, :], rhs=whh[:, k, ns],
                                         start=(k == 0), stop=(k == KH - 1))

                # ---- local top-2 (distinct ids) from the 24 pooled candidates;
                # each candidate's W_out row gather is issued as soon as its id
                # is known so the transfer overlaps the next selection round ----
                # top-2 by the sorted pooled values: slot k keeps entries in
                # the value band [m8l[k], m8l[k-1]) via (v is_lt m8l[k])*BIG +
                # (v is_ge m8l[k-1])*BIG + id, then min-reduce. Both chains
                # depend only on m8l, so the candidate-0 row gather issues
                # while the candidate-1 selection still runs. (Exact f32 value
                # ties across distinct ids are ~impossible.)
                tv = m8all[:, :, 0:TPT]
                nc.vector.tensor_copy(tif[:], i8all[:, :, 0:TPT])
                nc.vector.tensor_tensor(tif[:], tif[:], tbase[:].rearrange("b (n c) -> b n c", c=TPT), op=OP.add)
                nc.vector.max(out=m8l[:], in_=tv)
                for k in range(NCAND):
                    nc.vector.tensor_scalar(dcand[:], tv, m8l[:, k : k + 1], BIG, op0=OP.is_lt, op1=OP.mult)
                    if k > 0:
                        nc.vector.tensor_scalar(dhi[:], tv, m8l[:, k - 1 : k], BIG, op0=OP.is_ge, op1=OP.mult)
                        nc.vector.tensor_tensor(dcand[:], dcand[:], dhi[:], op=OP.add)
                    nc.vector.tensor_tensor(dcand[:], tif[:], dcand[:], op=OP.add)
                    nc.vector.tensor_reduce(si[:, k : k + 1], dcand[:], axis=AX.XY, op=OP.min)
                    nc.vector.tensor_copy(cu[:, k : k + 1], si[:, k : k + 1])
                    nc.gpsimd.indirect_dma_start(
                        out=wrow3[:, k], out_offset=None, in_=wfullb_p[:],
                        in_offset=bass.IndirectOffsetOnAxis(ap=cu[:, k : k + 1], axis=0),
                    )
                nc.vector.tensor_copy(pk[:, NCAND : 2 * NCAND], si[:])

                # ---- exact fp32 recheck of the candidates; per-candidate
                # reduce so candidate 0's dot finishes while candidate 1's
                # row gather is still in flight ----
                for j in range(NCAND):
                    nc.vector.tensor_tensor(prod3[:, j], wrow3[:, j], h1a[:], op=OP.mult)
                    nc.vector.tensor_reduce(pk[:, j : j + 1], prod3[:, j], axis=AX.X, op=OP.add)

                # ---- AllGather of (2 exact vals | 2 ids) per core ----
                nc.vector.transpose(pkT[:], pk[:])
                agin = dp.tile([2 * NCAND, B], f32, tag="agin")
                nc.sync.dma_start(out=agin[:, 0:32], in_=pkT[0 : 2 * NCAND, 0:32])
                nc.scalar.dma_start(out=agin[:, 32:64], in_=pkT[32 : 32 + 2 * NCAND, 0:32])
                # the bf16 logits write is issued on Sync here so most of its
                # transfers ride inside the collective window
                nc.sync.dma_start(out=out_p[t], in_=outb[:])
                agout = dp.tile([NCORES * 2 * NCAND, B], f32, tag="agout", addr_space="Shared")
                nc.gpsimd.collective_compute(
                    "AllGather", OP.bypass, replica_groups=rg,
                    ins=[agin[:].opt()], outs=[agout[:].opt()],
                )
                # block-swapped readback so one DVE stream-transpose finishes it
                nc.sync.dma_start(out=gpre[0:32, 0:32], in_=agout[0:32, 0:32])
                nc.scalar.dma_start(out=gpre[32:64, 0:32], in_=agout[0:32, 32:64])
                nc.vector.transpose(gpT[:], gpre[:])
                gall = gpT[:, 0:32].rearrange("b (r s) -> b r s", s=2 * NCAND)
                gv = gall[:, :, 0:NCAND]
                gi = gall[:, :, NCAND : 2 * NCAND]

                # ---- global argmax over 16 exact candidates (min-id tiebreak) ----
                nc.vector.max(out=gm8[:], in_=gv)
                nc.vector.tensor_scalar(msk[:], gv, gm8[:, 0:1], BIG, op0=OP.is_lt, op1=OP.mult)
                nc.vector.tensor_tensor(msk[:], gi, msk[:], op=OP.add)
                nc.vector.tensor_reduce(gidxf[:], msk[:], axis=AX.XY, op=OP.min)
                nc.vector.tensor_copy(gidx[:], gidxf[:])

                # ---- gather the token's precomputed gate row in two halves:
                # [f,g] lands first so the pointwise adds start early ----
                nc.gpsimd.indirect_dma_start(
                    out=gx[:, 0:HG], out_offset=None, in_=whe0_p[:],
                    in_offset=bass.IndirectOffsetOnAxis(ap=gidx[:, :1], axis=0),
                )
                nc.gpsimd.indirect_dma_start(
                    out=gx[:, HG:G4], out_offset=None, in_=whe1_p[:],
                    in_offset=bass.IndirectOffsetOnAxis(ap=gidx[:, :1], axis=0),
                )

    nc.finalize()  # Bacc: runs compile() legalization passes
    return nc


# gate-unit permutation [f, g, i, o] (torch order in the weights is i, f, g, o)
_PERM = np.concatenate([np.arange(512, 1024), np.arange(1024, 1536),
                        np.arange(0, 512), np.arange(1536, 2048)])


def make_in_maps(inputs):
    inp = {k: np.asarray(v) for k, v in inputs.items()}
    h0 = inp["h0"].astype(np.float32)
    c0 = inp["c0"].astype(np.float32)
    W_ih = inp["W_ih"].astype(np.float32)
    W_hh = inp["W_hh"].astype(np.float32)
    b = (inp["b_ih"].astype(np.float32) + inp["b_hh"].astype(np.float32))
    W_out = inp["W_out"].astype(np.float32)
    b_out = inp["b_out"].astype(np.float32)
    emb = inp["embed_table"].astype(np.float32)
    # x @ W_ih.T + b for every vocab row, fp32, gate units permuted to [f,g,i,o]
    whe = np.ascontiguousarray((emb @ W_ih.T + b)[:, _PERM].astype(np.float32))
    whe0 = np.ascontiguousarray(whe[:, 0 : G4 // 2])
    whe1 = np.ascontiguousarray(whe[:, G4 // 2 : G4])
    wfullb = np.ascontiguousarray(
        np.concatenate([W_out, b_out.reshape(V, 1)], axis=1).astype(np.float32))
    whhT = np.ascontiguousarray(W_hh[_PERM].T)
    bias = np.ascontiguousarray(b[_PERM].reshape(1, G4))
    in_maps = []
    for c in range(NCORES):
        base = c * VS
        tbase = np.zeros((B, NT * TPT), np.float32)
        for n in range(NT):
            tbase[:, TPT * n : TPT * (n + 1)] = float(base + n * TN)
        in_maps.append({
            "h0T": np.ascontiguousarray(h0.T),
            "c0": np.ascontiguousarray(c0),
            "whhT": whhT,
            "bias": bias,
            "woutT": np.ascontiguousarray(W_out[base : base + VS].T),
            "bout": np.ascontiguousarray(b_out[base : base + VS].reshape(1, VS)),
            "whe0": whe0,
            "whe1": whe1,
            "wfullb": wfullb,
            "tbase": tbase,
        })
    return in_maps


def run(inputs, steps=S, trace=False):
    from concourse.bass_utils import run_bass_kernel_spmd

    nc = build_program(steps)
    res = run_bass_kernel_spmd(nc, make_in_maps(inputs), list(range(NCORES)),
                               trace=trace)
    outs = [np.asarray(res.results[c]["out"]).astype(np.float32)
            for c in range(NCORES)]                            # each [steps, B, VS]
    full = np.concatenate(outs, axis=2)                        # [steps, B, V]
    return np.ascontiguousarray(np.transpose(full, (1, 0, 2))), res


def kernel(**inputs):
    out, _ = run(inputs, steps=S, trace=False)
    return out.astype(np.float32)
